# revision 1
# baseline (speedup 1.0000x reference)
"""NetVLAD-style vq_codebook kernel for 8 Trainium2 NeuronCores.

Reference computation (per full input):
  assn = BN(x @ clusters); softmax over 80 clusters, drop 16 ghosts
  vlad[b,d,k] = sum_n assn[b,n,k] x[b,n,d] - a_sum[b,k]*clusters2[d,k]
  intra-normalize over d, flatten, global L2 normalize -> (B, D*K)

Sharding: data-parallel over batch B (B/8 batches per core). BatchNorm
statistics (sum and sum-of-squares per cluster column, 2*80 floats) are
all-reduced across the 8 cores. Everything else is local.

Schedule (per core):
  Phase A (DMA-paced, ~52us floor): 16 groups of 4 token tiles; per group
    a cast-DMA load of x (fp32->fp16), an XBAR transpose to d-partition
    layout, 16 assignment matmuls, PSUM->SBUF copy of the logits, an f16
    square, and two accumulating BN-stats ones-matmuls (lagged 2 groups so
    the PE never stalls on the DVE square).
  Barrier: stats DRAM round-trip (stands in for / carries the AllReduce),
    BN affine chain, broadcast of scale|shift to all partitions.
  Phase B: per batch softmax (DVE muls + one Exp + row-sum + recip; the
    renormalize multiply is split DVE/Act to balance engines), then the
    vlad matmul with x stationary, a_sum ones-matmuls, and the
    a_sum*clusters2 correction on GPSIMD.
  Tail (batched over the 4 local batches): squares, intra-norm ones-
    matmuls, rsqrt chain. The reference's global L2 norm over the
    flattened, intra-normalized vlad is exactly sqrt(K)=8 (every column
    has unit norm), so it is folded in as a constant 1/8 scale.
"""

import sys

for _p in ("/opt/trn_rl_repo", "/root/.axon_site/_ro/trn_rl_repo"):
    if _p not in sys.path:
        sys.path.insert(0, _p)

import numpy as np

import concourse.bacc as bacc
import concourse.mybir as mybir
import concourse.tile as tile
from concourse.bass_utils import run_bass_kernel_spmd

F32 = mybir.dt.float32
F16 = mybir.dt.float16
AX = mybir.AxisListType
OP = mybir.AluOpType
ACTF = mybir.ActivationFunctionType

N_CORES = 8
D = 512
KG = 80          # clusters + ghosts
K = 64           # real clusters
N_SEQ = 2048
TPB = N_SEQ // 128   # token tiles per batch = 16
BN_EPS = 1e-5
L2_EPS = 1e-12


def build(b_loc=4, n_cores=N_CORES, with_collective=True, debug_taps=False):
    """Build the per-core program. b_loc = batches per core."""
    nt = b_loc * TPB                # token tiles per core = 64
    tok = nt * 128                  # tokens per core
    total_tok = tok * n_cores       # global token count for BN stats
    ngrp = nt // 4                  # 4-tile DMA/compute groups

    nc = bacc.Bacc("TRN2", target_bir_lowering=False, debug=False,
                   dynamic_dma_scratch_size=32768)

    x = nc.declare_dram_parameter("x", [tok, D], F32, isOutput=False)
    cl = nc.declare_dram_parameter("clusters", [D, KG], F32, isOutput=False)
    c2 = nc.declare_dram_parameter("clusters2", [D, K], F32, isOutput=False)
    gam = nc.declare_dram_parameter("bn_gamma", [1, KG], F32, isOutput=False)
    bet = nc.declare_dram_parameter("bn_beta", [1, KG], F32, isOutput=False)
    y = nc.declare_dram_parameter("y", [b_loc, D * K], F32, isOutput=True)
    if debug_taps:
        nt_ = b_loc * TPB
        dbg_aq = nc.declare_dram_parameter(
            "dbg_aq", [128, nt_ * 2 * KG], F32, isOutput=True)
        dbg_sm = nc.declare_dram_parameter(
            "dbg_sm", [128, nt_ * K], F32, isOutput=True)
        dbg_st = nc.declare_dram_parameter(
            "dbg_st", [1, 2 * KG], F32, isOutput=True)
        dbg_vv = nc.declare_dram_parameter(
            "dbg_vv", [128, b_loc * 4 * K], F32, isOutput=True)
        dbg_xh = nc.declare_dram_parameter(
            "dbg_xh", [128, 2 * TPB * D], F32, isOutput=True)
        dbg_cl = nc.declare_dram_parameter(
            "dbg_cl", [128, 4 * KG], F32, isOutput=True)

    ones_row_c = nc.inline_tensor(np.ones((1, 128), np.float32), name="c_ones_row")

    with tile.TileContext(nc) as tc:
        with (
            tc.tile_pool(name="persist", bufs=1) as persist,
            tc.tile_pool(name="work", bufs=4) as work,
            tc.tile_pool(name="dram", bufs=1, space="DRAM") as dram,
        ):
            # ---- persistent SBUF tensors ----
            # x lives in two tiles of two batches each; each tile has
            # exactly one DMA writer (a single big tile makes the
            # dependency tracker collapse subtile intervals and serialize
            # loads behind transposes, and >2 SWDGE loads in flight pick
            # up scheduler-inserted waits on unrelated transposes)
            xht = [persist.tile([128, 2, TPB, D], F16, name=f"xh{h}")
                   for h in range(b_loc // 2)]
            # interleaved logits|squares so one stats matmul covers both
            aq = persist.tile([128, nt, 2, KG], F16, name="aq")
            sm = persist.tile([128, nt, K], F16, name="sm")
            clh = persist.tile([128, 4, KG], F16, name="clh")
            c2n = persist.tile([128, 4, K], F32, name="c2n")
            ones16 = persist.tile([128, 1], F16, name="ones16")
            ones32 = persist.tile([128, 1], F32, name="ones32")
            epsc = persist.tile([1, 1], F32, name="epsc")
            ones_row = persist.tile([1, 128], F32, name="ones_row")
            gamma = persist.tile([1, KG], F32, name="gamma")
            beta = persist.tile([1, KG], F32, name="beta")
            stats_sb = persist.tile([1, 2 * KG], F32, name="stats_sb")
            stats_g = persist.tile([1, 2 * KG], F32, name="stats_g")
            ss = persist.tile([1, 2 * KG], F32, name="ss")
            bcB = persist.tile([128, 2, KG], F16, name="bcB")
            vv = persist.tile([128, b_loc, 4, K], F32, name="vv")
            vf = persist.tile([128, b_loc, 4, K], F32, name="vf")
            pa_sb = persist.tile([1, b_loc, K], F32, name="pa_sb")
            dummy = persist.tile([1, 1], F32, name="dummy")

            stats_in = dram.tile([1, 2 * KG], F32, name="stats_in")
            stats_out = dram.tile([1, 2 * KG], F32, name="stats_out")


            # ---- phase A: load + transpose + assignment + BN stats ----
            # Token permutation: xh[b][p, t] = x[b, 16*p + t] with
            # partition p holding 16 consecutive tokens of each batch
            # ("(b p t) d" order). Tokens are exchangeable within a batch
            # (BN stats, softmax, vlad, a_sum are all order-invariant);
            # this makes every partition's HBM read contiguous, so each
            # batch loads with ~128 DMA descriptors in one instruction.
            # DMA instructions carry a fixed ~2-3us serial launch cost in
            # the scheduler, so phase A uses as few as possible: 4 batch
            # loads + 4 batch XBAR transposes. ALL loads are emitted
            # before any transpose (a load emitted after a transpose picks
            # up a false WAR edge and serializes).
            # constants first (small, on the SWDGE/scalar queues) so the
            # transpose stream on the sync queue is never interrupted;
            # preload the Sqrt act table while the Act engine is idle
            nc.scalar.dma_start(ones_row[:], ones_row_c.ap()[:, :])
            nc.scalar.dma_start(gamma[:], gam[:, :])
            nc.scalar.dma_start(beta[:], bet[:, :])
            nc.gpsimd.dma_start(
                clh[:], cl.ap().rearrange("(c p) k -> p c k", p=128))
            nc.gpsimd.dma_start(
                c2n[:], c2.ap().rearrange("(c p) k -> p c k", p=128))
            nc.vector.memset(ones16[:], 1.0)
            nc.vector.memset(ones32[:], 1.0)
            nc.vector.memset(epsc[:], BN_EPS)
            nc.vector.memset(dummy[:], 1.0)
            nc.scalar.sqrt(dummy[:], dummy[:])
            xr = x.ap().rearrange("(b p t) d -> p b (t d)", p=128, t=TPB)
            for h in range(b_loc // 2):
                nc.gpsimd.dma_start(
                    xht[h][:].rearrange("p b t d -> p b (t d)"),
                    xr[:, 2 * h:2 * h + 2, :])
            with tc.tile_pool(name="psA", bufs=1, space="PSUM") as psA:
                pstat = psA.tile([1, 2, 2, KG], F32, name="pstat",
                                 tag="pstat", bufs=1)

                def emit_stats(q):
                    # accumulating [1, 2*2*KG=320] ones-matmuls per tile pair
                    for h in range(2):
                        s0 = 4 * q + 2 * h
                        nc.tensor.matmul(
                            pstat[:], ones16[:], aq[:, s0:s0 + 2, :, :],
                            start=(s0 == 0), stop=(s0 == nt - 2),
                            skip_group_check=True)

                # 8-tile (4096-column) slabs. On hardware the XBAR
                # transpose's completion semaphore fires when the ucode is
                # dispatched, not when the data lands, so a consumer keyed
                # on it races the transpose. All transposes go on ONE DGE
                # queue (FIFO): a transpose's dispatch therefore implies
                # the previous transpose's data is fully written. Each
                # slab's assignment matmuls are fenced by a guard matmul
                # that reads slot 0 of the NEXT slab's transpose; the last
                # slab is fenced by a small same-queue self-copy instead.
                slabs = [(t0s // TPB, t0s, 8) for t0s in range(0, nt, 8)]
                xhTgs = []

                def emit_slab(sj):
                    bj, t0j, nj = slabs[sj]
                    xhT = xhTgs[sj]
                    for q in range(nj // 4):
                        p1 = psA.tile([128, 4, KG], F32, name="p1",
                                      tag="p1", bufs=3)
                        for j in range(4):
                            for c in range(4):
                                nc.tensor.matmul(
                                    p1[:, j, :],
                                    xhT[:, 16 * q + 4 * j + c, :],
                                    clh[:, c, :], start=(c == 0),
                                    stop=(c == 3), skip_group_check=True)
                        # stats lag a chunk behind so the PE never waits
                        # on the DVE square of the current chunk
                        qq = t0j // 4 + q
                        if qq >= 1:
                            emit_stats(qq - 1)
                        h0 = 4 * qq
                        nc.vector.tensor_copy(aq[:, h0:h0 + 4, 0, :],
                                              p1[:])
                        with nc.allow_low_precision("fp16 logit squares"):
                            nc.vector.tensor_tensor(
                                aq[:, h0:h0 + 4, 1, :],
                                aq[:, h0:h0 + 4, 0, :],
                                aq[:, h0:h0 + 4, 0, :], op=OP.mult)

                for si, (b, t0s, ntl) in enumerate(slabs):
                    xhTg = work.tile([128, 32, 128], F16, name="xhTg",
                                     tag="xhT", bufs=4)
                    # XBAR transpose: (128, ntl*512) -> (128, 4*ntl, 128)
                    # with logical row 128*e + p at [:, e, :]; e = 4*j + c,
                    # d = 128*c + p (chunk-major per tile), matching clh
                    toff = t0s - TPB * b
                    nc.sync.dma_start(
                        xhTg[:, :4 * ntl, :],
                        xht[b // 2][:, b % 2, toff:toff + ntl, :],
                        transpose=True)
                    xhTgs.append(xhTg)
                    if si >= 1:
                        pguard = psA.tile([128, 1], F32, name="pguard",
                                          tag="guard", bufs=2)
                        nc.tensor.matmul(pguard[:], xhTg[:, 0, :],
                                         ones16[:], start=True, stop=True,
                                         skip_group_check=True)
                        emit_slab(si - 1)
                # last slab: same-queue self-copy guard (the data dep
                # on slot 31 pins it after the transpose in queue order)
                nc.sync.dma_start(xhTgs[-1][:, 31, :], xhTgs[-1][:, 31, :])
                pguard = psA.tile([128, 1], F32, name="pguard",
                                  tag="guard", bufs=2)
                nc.tensor.matmul(pguard[:], xhTgs[-1][:, 31, :], ones16[:],
                                 start=True, stop=True,
                                 skip_group_check=True)
                emit_slab(len(slabs) - 1)
                emit_stats(nt // 4 - 1)

                # fold the tile-pair axis: [1,(i,s,k)] -> [1,(s,k)]
                nc.vector.tensor_reduce(
                    stats_sb[:].rearrange("p (s k) -> p s k", s=2),
                    pstat[:].rearrange("p i s k -> p s k i"),
                    axis=AX.X, op=OP.add)

            # ---- all-reduce stats ----
            nc.sync.dma_start(stats_in[:], stats_sb[:])
            if with_collective:
                nc.gpsimd.collective_compute(
                    "AllReduce", OP.add,
                    replica_groups=[list(range(n_cores))],
                    ins=[stats_in.opt()], outs=[stats_out.opt()])
            else:
                nc.sync.dma_start(stats_out[:], stats_in[:])
            nc.sync.dma_start(stats_g[:], stats_out[:])

            # ---- BN affine: scale = gamma*rsqrt(var+eps); shift = beta-mean*scale
            mq = work.tile([1, 2 * KG], F32, name="mq", tag="sv", bufs=6)
            msq = work.tile([1, KG], F32, name="msq", tag="sv", bufs=6)
            var = work.tile([1, KG], F32, name="var", tag="sv", bufs=6)
            sd = work.tile([1, KG], F32, name="sd", tag="sv", bufs=6)
            rsd = work.tile([1, KG], F32, name="rsd", tag="sv", bufs=6)
            t1 = work.tile([1, KG], F32, name="t1", tag="sv", bufs=6)
            inv_n = 1.0 / float(total_tok)
            nc.vector.tensor_scalar_mul(mq[:], stats_g[:], inv_n)
            nc.vector.tensor_tensor(msq[:], mq[:, :KG], mq[:, :KG], op=OP.mult)
            nc.vector.tensor_tensor(var[:], mq[:, KG:], msq[:], op=OP.subtract)
            nc.scalar.activation(sd[:], var[:], ACTF.Sqrt, bias=epsc[:])
            nc.vector.reciprocal(rsd[:], sd[:])
            nc.vector.tensor_tensor(ss[:, :KG], rsd[:], gamma[:], op=OP.mult)
            nc.vector.tensor_tensor(t1[:], mq[:, :KG], ss[:, :KG], op=OP.mult)
            nc.vector.tensor_tensor(ss[:, KG:], beta[:], t1[:], op=OP.subtract)

            # ---- phase B: softmax + vlad + normalization ----
            with tc.tile_pool(name="psB", bufs=1, space="PSUM") as psB:
                pbc = psB.tile([128, 2 * KG], F32, name="pbc",
                               tag="misc", bufs=2)
                nc.tensor.matmul(pbc[:], ones_row[:], ss[:], start=True,
                                 stop=True, skip_group_check=True)
                nc.vector.tensor_copy(bcB[:].rearrange("p s k -> p (s k)"),
                                      pbc[:])
                scale_b = bcB[:, 0:1, :]
                shift_b = bcB[:, 1:2, :]

                te_tiles = {}

                def s1a(b):
                    # te = assn*scale + shift; exp. For b0 the ops are
                    # emitted in two half-batch pieces so the first
                    # exp/denominator can start ~1.5us earlier (pipeline
                    # fill); later batches overlap and use one piece.
                    t0 = b * TPB
                    te = work.tile([128, TPB, KG], F16, name="te",
                                   tag="te", bufs=2)
                    pieces = ((0, TPB // 2), (TPB // 2, TPB)) if b == 0 \
                        else ((0, TPB),)
                    for (ta, tb) in pieces:
                        n = tb - ta
                        nc.vector.tensor_tensor(
                            te[:, ta:tb], aq[:, t0 + ta:t0 + tb, 0, :],
                            scale_b.to_broadcast([128, n, KG]), op=OP.mult)
                        nc.vector.tensor_tensor(
                            te[:, ta:tb], te[:, ta:tb],
                            shift_b.to_broadcast([128, n, KG]), op=OP.add)
                        nc.scalar.activation(te[:, ta:tb], te[:, ta:tb],
                                             ACTF.Exp)
                    te_tiles[b] = te

                def s1b(b):
                    # denominators + renormalize; renorm split DVE/Act
                    # (DVE produces the first half, which the vlad matmul
                    # consumes first). b0 is emitted per half so its first
                    # sm tiles are ready ~1.5us sooner (pipeline fill).
                    t0 = b * TPB
                    te = te_tiles.pop(b)
                    half = TPB // 2
                    denom = work.tile([128, TPB], F16, name="denom",
                                      tag="dn", bufs=2)
                    # fp32: the Act Copy's scale AP must be FP32
                    recip = work.tile([128, TPB], F32, name="recip",
                                      tag="rc", bufs=2)
                    pieces = ((0, half), (half, TPB)) if b == 0 \
                        else ((0, TPB),)
                    for (ta, tb) in pieces:
                        with nc.allow_low_precision("fp16 softmax denom"):
                            nc.vector.tensor_reduce(
                                denom[:, ta:tb], te[:, ta:tb], axis=AX.X,
                                op=OP.add)
                        nc.vector.reciprocal(recip[:, ta:tb],
                                              denom[:, ta:tb])
                        mid = min(tb, max(ta, half))
                        if mid > ta:
                            nc.vector.tensor_tensor(
                                sm[:, t0 + ta:t0 + mid, :],
                                te[:, ta:mid, :K],
                                recip[:, ta:mid]
                                .rearrange("p (t a) -> p t a", a=1)
                                .to_broadcast([128, mid - ta, K]),
                                op=OP.mult)
                        if tb > mid:
                            with nc.allow_low_precision("fp16 softmax"):
                                nc.gpsimd.tensor_tensor(
                                    sm[:, t0 + mid:t0 + tb, :],
                                    te[:, mid:tb, :K],
                                    recip[:, mid:tb]
                                    .rearrange("p (t a) -> p t a", a=1)
                                    .to_broadcast([128, tb - mid, K]),
                                    op=OP.mult)

                def s2(b):
                    # vlad matmul with x stationary; the a_sum ones-matmuls
                    # go after the first chunk pass (all sm tiles consumed
                    # by then) so s3's pam/av overlap the remaining chunks
                    t0 = b * TPB
                    pas = psB.tile([1, K], F32, name="pas",
                                   tag="pas", bufs=2)
                    pv = psB.tile([128, 4, K], F32, name="pv",
                                  tag="pv", bufs=3)
                    # NOTE: groups must be contiguous per PSUM bank region --
                    # start=True clears has_written for the whole bank
                    for c in range(4):
                        for i in range(TPB):
                            t = t0 + i
                            nc.tensor.matmul(
                                pv[:, c, :],
                                xht[b // 2][:, b % 2, i,
                                            c * 128:(c + 1) * 128],
                                sm[:, t, :],
                                start=(i == 0), stop=(i == TPB - 1),
                                skip_group_check=True)
                        if c == 0:
                            for u in range(TPB):
                                nc.tensor.matmul(
                                    pas[:], ones16[:], sm[:, t0 + u, :],
                                    start=(u == 0), stop=(u == TPB - 1),
                                    skip_group_check=True)
                    return pv, pas

                def s3(b, pv, pas):
                    # pa_sb copy emitted here (one batch late) so the DVE
                    # queue never blocks on the PE's pas accumulation
                    nc.vector.tensor_copy(pa_sb[:, b, :], pas[:])
                    pam = psB.tile([128, K], F32, name="pam",
                                   tag="pam", bufs=1)
                    nc.tensor.matmul(pam[:], ones_row[:], pa_sb[:, b, :],
                                     start=True, stop=True,
                                     skip_group_check=True)
                    # GPSIMD cannot read PSUM on hardware: copy the pam
                    # broadcast to SBUF (DVE), compute av on GPSIMD, and do
                    # the pv subtraction on DVE
                    pam_sb = work.tile([128, K], F32, name="pam_sb",
                                       tag="pams", bufs=2)
                    nc.vector.tensor_copy(pam_sb[:], pam[:])
                    av = work.tile([128, 4, K], F32, name="av",
                                   tag="av", bufs=2)
                    nc.gpsimd.tensor_tensor(
                        av[:], c2n[:],
                        pam_sb[:].rearrange("p (a k) -> p a k", a=1)
                        .to_broadcast([128, 4, K]), op=OP.mult)
                    nc.vector.tensor_tensor(vv[:, b], pv[:], av[:],
                                            op=OP.subtract)

                yb = y.ap().rearrange("b (c p k) -> p b c k", p=128, k=K)

                def tail_one(b):
                    # normalization for batch b: intra-norm 1/||v||; the
                    # global norm of the flattened intra-normalized vlad is
                    # exactly sqrt(K)=8 -> fold 1/8. n2 comes from an fp32
                    # matmul of vv against itself per chunk (diag not
                    # needed: lhsT=vv chunk, rhs=vv chunk gives k x k; too
                    # big) -- instead square on the PE is not possible, so
                    # n2 = ones^T (vv*vv) still needs the elementwise
                    # square; to keep it off the DVE it runs as an fp32
                    # matmul with vv as BOTH stationary and moving is
                    # invalid, so: square on GPSIMD (idle here), fp32
                    # ones-matmul reduce, no DVE reduce.
                    sq = work.tile([128, 4, K], F32, name="sq",
                                   tag="sq", bufs=2)
                    nc.gpsimd.tensor_tensor(sq[:], vv[:, b], vv[:, b],
                                            op=OP.mult)
                    pnrm = psB.tile([1, 4, K], F32, name="pnrm",
                                    tag="misc", bufs=2)
                    nc.tensor.matmul(pnrm[:], ones32[:], sq[:], start=True,
                                     stop=True, skip_group_check=True)
                    n2 = work.tile([1, K], F32, name="n2", tag="n2",
                                   bufs=6)
                    nc.vector.tensor_reduce(
                        n2[:], pnrm[:].rearrange("p c k -> p k c"),
                        axis=AX.X, op=OP.add)
                    snorm = work.tile([1, K], F32, name="snorm",
                                      tag="n2", bufs=6)
                    nc.scalar.activation(snorm[:], n2[:], ACTF.Sqrt)
                    nc.vector.tensor_scalar(snorm[:], snorm[:], L2_EPS, 8.0,
                                            op0=OP.max, op1=OP.mult)
                    rn = work.tile([1, K], F32, name="rn", tag="n2",
                                   bufs=6)
                    nc.vector.reciprocal(rn[:], snorm[:])
                    prnB = psB.tile([128, K], F32, name="prnB",
                                    tag="misc", bufs=2)
                    nc.tensor.matmul(prnB[:], ones_row[:], rn[:],
                                     start=True, stop=True,
                                     skip_group_check=True)
                    nc.vector.tensor_tensor(
                        vf[:, b], vv[:, b],
                        prnB[:].rearrange("p (a k) -> p a k", a=1)
                        .to_broadcast([128, 4, K]), op=OP.mult)
                    dma_eng = nc.sync if b % 2 == 0 else nc.scalar
                    dma_eng.dma_start(yb[:, b], vf[:, b])

                # software pipeline: s1a one batch ahead; s3 lags one batch
                # so the PE's pam matmul never waits on the DVE reduce;
                # the first tail half (b0,b1) overlaps the b3 vlad work
                s1a(0)
                pvs = {}
                for b in range(b_loc):
                    if b + 1 < b_loc:
                        s1a(b + 1)
                    s1b(b)
                    pvs[b] = s2(b)
                    if b >= 1:
                        s3(b - 1, *pvs.pop(b - 1))
                        tail_one(b - 1)
                s3(b_loc - 1, *pvs.pop(b_loc - 1))
                tail_one(b_loc - 1)
                if debug_taps:
                    nc.gpsimd.dma_start(
                        dbg_aq.ap().rearrange("p (t s k) -> p t s k",
                                              t=nt, s=2), aq[:])
                    nc.gpsimd.dma_start(
                        dbg_sm.ap().rearrange("p (t k) -> p t k", t=nt),
                        sm[:])
                    nc.sync.dma_start(dbg_st.ap()[:, :], stats_g[:])
                    nc.sync.dma_start(
                        dbg_vv.ap().rearrange("p (b c k) -> p b c k",
                                              b=b_loc, c=4), vv[:])
                    nc.gpsimd.dma_start(
                        dbg_xh.ap().rearrange("p (b t d) -> p b t d",
                                              b=2, t=TPB), xht[0][:])
                    nc.gpsimd.dma_start(
                        dbg_cl.ap().rearrange("p (c k) -> p c k", c=4),
                        clh[:])
    nc.compile()
    return nc


_CACHE = {}


def _get(b_loc, n_cores, with_collective):
    key = (b_loc, n_cores, with_collective)
    if key not in _CACHE:
        _CACHE[key] = build(b_loc, n_cores, with_collective)
    return _CACHE[key]


def make_in_maps(x, clusters, clusters2, bn_gamma, bn_beta, n_cores=N_CORES):
    B = x.shape[0]
    b_loc = B // n_cores
    shared = {
        "clusters": np.ascontiguousarray(clusters, np.float32),
        "clusters2": np.ascontiguousarray(
            np.asarray(clusters2).reshape(D, K), np.float32),
        "bn_gamma": np.ascontiguousarray(
            np.asarray(bn_gamma).reshape(1, KG), np.float32),
        "bn_beta": np.ascontiguousarray(
            np.asarray(bn_beta).reshape(1, KG), np.float32),
    }
    in_maps = []
    for i in range(n_cores):
        m = dict(shared)
        m["x"] = np.ascontiguousarray(
            np.asarray(x[i * b_loc:(i + 1) * b_loc]).reshape(
                b_loc * N_SEQ, D), np.float32)
        in_maps.append(m)
    return in_maps


def kernel(x, clusters, clusters2, bn_gamma, bn_beta):
    B, N, Dd = x.shape
    assert (N, Dd) == (N_SEQ, D) and B % N_CORES == 0
    b_loc = B // N_CORES
    nc = _get(b_loc, N_CORES, True)
    in_maps = make_in_maps(x, clusters, clusters2, bn_gamma, bn_beta)
    res = run_bass_kernel_spmd(nc, in_maps, core_ids=list(range(N_CORES)))
    out = np.concatenate([res.results[i]["y"] for i in range(N_CORES)], axis=0)
    return out



# revision 30
# speedup vs baseline: 1.4141x; 1.4141x over previous
"""NetVLAD-style vq_codebook kernel for 8 Trainium2 NeuronCores.

Reference computation (per full input):
  assn = BN(x @ clusters); softmax over 80 clusters, drop 16 ghosts
  vlad[b,d,k] = sum_n assn[b,n,k] x[b,n,d] - a_sum[b,k]*clusters2[d,k]
  intra-normalize over d, flatten, global L2 normalize -> (B, D*K)

Sharding: data-parallel over batch B (B/8 batches per core). BatchNorm
statistics (sum and sum-of-squares per cluster column) are all-reduced
across the 8 cores. Everything else is local.

Schedule (per core):
  Phase A (x-load paced): 16 quarter-batch cast-DMA loads of x
    (fp32->f16, token-major). Per batch, the d-major copy of x needed by
    the assignment matmul is produced on the PE (transpose-mode matmuls
    into f16 PSUM banks, 8 chunk-tiles per bank) and copied to SBUF on
    DVE/Act -- keeping the serial DMA resource free for the loads
    (XBAR transposes would cost more DMA time than the loads). The
    assignment matmuls then run per 4-tile group; logits are copied
    PSUM->SBUF as f16 on Act into aqp (81 columns: 80 logits + a ones
    column). BN stats come from one PE matmul per tile accumulating
    G = [l|1]^T [l|1]: row 80 of G is sum(l), the diagonal is sum(l^2).
  Barrier: diagonal extraction + PE transpose to a [2,81] stats row,
    DRAM round-trip AllReduce, BN affine chain, one f16 broadcast
    matmul of scale|shift to all partitions.
  Phase B: per batch softmax (DVE scale/shift muls + one Exp on Act +
    row-sum + recip; renormalize split DVE/Pool), vlad matmul with x
    stationary, a_sum ones-matmuls, a_sum*clusters2 correction on
    GPSIMD. Tail: squares on GPSIMD, intra-norm via 4 accumulating
    ones-matmuls into one PSUM tile (no single-partition reduce),
    sqrt(64*n2) folds the exact global norm sqrt(K)=8.
"""

import sys

for _p in ("/opt/trn_rl_repo", "/root/.axon_site/_ro/trn_rl_repo"):
    if _p not in sys.path:
        sys.path.insert(0, _p)

import numpy as np

import concourse.bacc as bacc
import concourse.mybir as mybir
import concourse.tile as tile
from concourse.bass_utils import run_bass_kernel_spmd
from concourse.masks import make_identity

F32 = mybir.dt.float32
F16 = mybir.dt.float16
AX = mybir.AxisListType
OP = mybir.AluOpType
ACTF = mybir.ActivationFunctionType

N_CORES = 8
D = 512
KG = 80          # clusters + ghosts
K = 64           # real clusters
N_SEQ = 2048
TPB = N_SEQ // 128   # token tiles per batch = 16
QT = 4               # token tiles per x-load quarter
BN_EPS = 1e-5


def build(b_loc=4, n_cores=N_CORES, with_collective=True):
    """Build the per-core program. b_loc = batches per core."""
    nt = b_loc * TPB                # token tiles per core = 64
    tok = nt * 128                  # tokens per core
    total_tok = tok * n_cores       # global token count for BN stats

    nc = bacc.Bacc("TRN2", target_bir_lowering=False, debug=False,
                   dynamic_dma_scratch_size=32768)

    x = nc.declare_dram_parameter("x", [tok, D], F32, isOutput=False)
    cl = nc.declare_dram_parameter("clusters", [D, KG], F32, isOutput=False)
    c2 = nc.declare_dram_parameter("clusters2", [D, K], F32, isOutput=False)
    gam = nc.declare_dram_parameter("bn_gamma", [1, KG], F32, isOutput=False)
    bet = nc.declare_dram_parameter("bn_beta", [1, KG], F32, isOutput=False)
    y = nc.declare_dram_parameter("y", [b_loc, D * K], F32, isOutput=True)

    with tile.TileContext(nc) as tc:
        with (
            tc.tile_pool(name="persist", bufs=1) as persist,
            tc.tile_pool(name="work", bufs=4) as work,
            tc.tile_pool(name="dram", bufs=1, space="DRAM") as dram,
        ):
            # ---- persistent SBUF tensors ----
            # token-major x, one tile per load quarter (exactly one DMA
            # writer per tile keeps the dependency tracker exact)
            xq = [persist.tile([128, QT, D], F16, name=f"xq{i}")
                  for i in range(nt // QT)]
            # logits + ones column so one Gram matmul yields both BN sums
            aqp = persist.tile([128, nt, KG + 1], F16, name="aqp")
            sm = persist.tile([128, nt, K], F16, name="sm")
            clh = persist.tile([128, 4, KG], F16, name="clh")
            c2n = persist.tile([128, 4, K], F32, name="c2n")
            ones16 = persist.tile([128, 1], F16, name="ones16")
            ones_row16 = persist.tile([1, 128], F16, name="ones_row16")
            ident16 = persist.tile([128, 128], F16, name="ident16")
            ident81 = persist.tile([KG + 1, KG + 1], F32, name="ident81")
            epsc = persist.tile([1, 1], F32, name="epsc")
            gamma = persist.tile([1, KG], F32, name="gamma")
            beta = persist.tile([1, KG], F32, name="beta")
            scr2 = persist.tile([KG + 1, 2], F32, name="scr2")
            stats_sb = persist.tile([2, KG + 1], F32, name="stats_sb")
            stats_g = persist.tile([1, 2 * (KG + 1)], F32, name="stats_g")
            ss = persist.tile([1, 2 * KG], F32, name="ss")
            ssh = persist.tile([1, 2 * KG], F16, name="ssh")
            bcB = persist.tile([128, 2, KG], F16, name="bcB")
            vv = persist.tile([128, b_loc, 4, K], F32, name="vv")
            vf = persist.tile([128, b_loc, 4, K], F32, name="vf")
            pa_sb = persist.tile([1, b_loc, K], F16, name="pa_sb")
            dummy = persist.tile([1, 1], F32, name="dummy")

            stats_in = dram.tile([1, 2 * (KG + 1)], F32, name="stats_in")
            stats_out = dram.tile([1, 2 * (KG + 1)], F32, name="stats_out")

            # ---- x loads: 16 quarter-batch cast DMAs (only the SWDGE /
            # gpsimd queue can cast fp32->f16). Token permutation:
            # partition p holds tokens 16p..16p+15 of each batch
            # ("(b p t) d" order, order-invariant math), so every load is
            # 128 descriptors of 8KB contiguous HBM. The identities (also
            # gpsimd) slot in after the first two load issues: ready well
            # before the first transpose, without delaying the loads.
            xr = x.ap().rearrange("(b p t) d -> p b (t d)", p=128, t=TPB)

            def emit_xload(i):
                b, q = divmod(i, TPB // QT)
                nc.gpsimd.dma_start(
                    xq[i][:].rearrange("p t d -> p (t d)"),
                    xr[:, b, q * QT * D:(q + 1) * QT * D])

            emit_xload(0)
            emit_xload(1)
            make_identity(nc, ident16[:])
            make_identity(nc, ident81[:])
            for i in range(2, nt // QT):
                emit_xload(i)

            # ---- constants (small, off the gpsimd/SWDGE queue) ----
            clf = work.tile([128, 4, KG], F32, name="clf", tag="clf", bufs=1)
            nc.scalar.dma_start(gamma[:], gam[:, :])
            nc.scalar.dma_start(beta[:], bet[:, :])
            nc.scalar.dma_start(
                clf[:], cl.ap().rearrange("(c p) k -> p c k", p=128))
            nc.scalar.dma_start(
                c2n[:], c2.ap().rearrange("(c p) k -> p c k", p=128))
            with nc.allow_low_precision("fp16 clusters"):
                nc.vector.tensor_copy(clh[:], clf[:])
            nc.vector.memset(ones16[:], 1.0)
            nc.vector.memset(ones_row16[:], 1.0)
            nc.vector.memset(epsc[:], BN_EPS)
            # ones column of aqp (stride-81 writes)
            nc.vector.memset(aqp[:, :, KG:], 1.0)
            # preload Act tables (Sqrt + Exp) while Act is idle
            nc.vector.memset(dummy[:], 1.0)
            nc.scalar.sqrt(dummy[:], dummy[:])
            nc.scalar.activation(dummy[:], dummy[:], ACTF.Exp)

            with tc.tile_pool(name="psA", bufs=1, space="PSUM") as psA:
                g81 = psA.tile([KG + 1, KG + 1], F32, name="g81",
                               tag="g81", bufs=1)

                # PE p-state warmup: data-independent matmuls fill the
                # ramp window while the first x quarter is still in
                # flight, so the real transposes start at full clock
                for _w in range(14):
                    pw = psA.tile([128, 4, KG], F32, name="pw",
                                  tag="p1", bufs=3)
                    nc.tensor.matmul(pw[:, 0, :], ident16[:],
                                     ident16[:, :KG], start=True,
                                     stop=True, skip_group_check=True)

                gram_pend = []

                def emit_gram(flush=False):
                    # lag one group behind the aqp copies so the PE never
                    # stalls on the Act copy of the current group
                    while len(gram_pend) > (0 if flush else 4):
                        t = gram_pend.pop(0)
                        nc.tensor.matmul(
                            g81[:], aqp[:, t, :], aqp[:, t, :],
                            start=(t == 0), stop=(t == nt - 1),
                            skip_group_check=True)

                def assn_group(b, g):
                    # logits for tiles 4g..4g+3 of batch b
                    p1 = psA.tile([128, 4, KG], F32, name="p1",
                                  tag="p1", bufs=3)
                    for j in range(4):
                        t = 4 * g + j
                        for c in range(4):
                            nc.tensor.matmul(
                                p1[:, j, :],
                                xhT_cur[:, 4 * t + c, :],
                                clh[:, c, :], start=(c == 0),
                                stop=(c == 3), skip_group_check=True)
                    emit_gram()
                    t0 = TPB * b + 4 * g
                    nc.scalar.copy(aqp[:, t0:t0 + 4, :KG], p1[:])
                    gram_pend.extend(range(t0, t0 + 4))

                # Per batch: 64 PE transpose-matmuls (f16 -> PSUM, 8 chunk
                # tiles per bank), bank copies to SBUF on DVE/Act, then
                # the assignment matmuls in 4-tile groups interleaved so
                # the PE keeps running while copies drain.
                for b in range(b_loc):
                    xhT_cur = work.tile([128, 4 * TPB, 128], F16,
                                        name="xhT", tag="xhT", bufs=2)
                    for qb in range(8):          # 8 PSUM banks per batch
                        tp = psA.tile([128, 8, 128], F16, name="tp",
                                      tag="tp", bufs=4)
                        for j in range(2):       # 2 token tiles per bank
                            t = 2 * qb + j
                            src = xq[4 * b + t // QT]
                            for c in range(4):
                                nc.tensor.transpose(
                                    tp[:, 4 * j + c, :],
                                    src[:, t % QT, 128 * c:128 * (c + 1)],
                                    ident16[:])
                        # DVE copies are 1.6x faster; Act takes one bank
                        # per batch to keep DVE under the load pace
                        on_act = qb == 4
                        if on_act:
                            nc.scalar.copy(
                                xhT_cur[:, 8 * qb:8 * qb + 8, :], tp[:])
                        else:
                            nc.vector.tensor_copy(
                                xhT_cur[:, 8 * qb:8 * qb + 8, :], tp[:])
                        if qb == 3:
                            assn_group(b, 0)
                        elif qb == 5:
                            assn_group(b, 1)
                        elif qb == 7:
                            assn_group(b, 2)
                    assn_group(b, 3)
                emit_gram(flush=True)

                # ---- BN stats: diag(G) = sum(l^2), row 80 = sum(l) ----
                g_scr = work.tile([KG + 1, KG + 1], F32, name="g_scr",
                                  tag="gscr", bufs=1)
                nc.vector.scalar_tensor_tensor(
                    g_scr[:], g81[:], 1.0, ident81[:],
                    op0=OP.bypass, op1=OP.mult, accum_out=scr2[:, 0:1])
                nc.vector.tensor_copy(scr2[:, 1:2], g81[:, KG:])

            with tc.tile_pool(name="psB", bufs=1, space="PSUM") as psB:
                pT = psB.tile([2, KG + 1], F32, name="pT",
                              tag="misc", bufs=2)
                nc.tensor.matmul(pT[:], scr2[:], ident81[:],
                                 is_transpose=True, start=True, stop=True,
                                 skip_group_check=True)
                nc.vector.tensor_copy(stats_sb[:], pT[:])

                # ---- all-reduce stats (row0 = sum l^2 | row1 = sum l) ----
                nc.sync.dma_start(
                    stats_in[:].rearrange("a (p k) -> p (a k)", p=2),
                    stats_sb[:])
                if with_collective:
                    nc.gpsimd.collective_compute(
                        "AllReduce", OP.add,
                        replica_groups=[list(range(n_cores))],
                        ins=[stats_in.opt()], outs=[stats_out.opt()])
                else:
                    nc.sync.dma_start(stats_out[:], stats_in[:])
                nc.sync.dma_start(stats_g[:], stats_out[:])

                # ---- BN affine: scale = gamma*rsqrt(var+eps);
                #      shift = beta - mean*scale
                KG1 = KG + 1
                mean = work.tile([1, KG], F32, name="mean", tag="sv",
                                 bufs=6)
                msq = work.tile([1, KG], F32, name="msq", tag="sv", bufs=6)
                var = work.tile([1, KG], F32, name="var", tag="sv", bufs=6)
                sd = work.tile([1, KG], F32, name="sd", tag="sv", bufs=6)
                rsd = work.tile([1, KG], F32, name="rsd", tag="sv", bufs=6)
                t1 = work.tile([1, KG], F32, name="t1", tag="sv", bufs=6)
                inv_n = 1.0 / float(total_tok)
                nc.vector.tensor_scalar_mul(mean[:], stats_g[:, KG1:KG1 + KG],
                                            inv_n)
                nc.vector.tensor_tensor(msq[:], mean[:], mean[:], op=OP.mult)
                # var = sum(l^2)*invN - mean^2 in one fused op
                nc.vector.scalar_tensor_tensor(
                    var[:], stats_g[:, 0:KG], inv_n, msq[:],
                    op0=OP.mult, op1=OP.subtract)
                nc.scalar.activation(sd[:], var[:], ACTF.Sqrt, bias=epsc[:])
                nc.vector.reciprocal(rsd[:], sd[:])
                nc.vector.tensor_tensor(ss[:, :KG], rsd[:], gamma[:],
                                        op=OP.mult)
                nc.vector.tensor_tensor(t1[:], mean[:], ss[:, :KG],
                                        op=OP.mult)
                nc.vector.tensor_tensor(ss[:, KG:], beta[:], t1[:],
                                        op=OP.subtract)
                with nc.allow_low_precision("fp16 bn scale/shift"):
                    nc.vector.tensor_copy(ssh[:], ss[:])

                pbc = psB.tile([128, 2 * KG], F32, name="pbc",
                               tag="misc", bufs=2)
                nc.tensor.matmul(pbc[:], ones_row16[:], ssh[:], start=True,
                                 stop=True, skip_group_check=True)
                nc.vector.tensor_copy(bcB[:].rearrange("p s k -> p (s k)"),
                                      pbc[:])
                scale_b = bcB[:, 0:1, :]
                shift_b = bcB[:, 1:2, :]

                yb = y.ap().rearrange("b (c p k) -> p b c k", p=128, k=K)
                te_tiles = {}
                pvs = {}
                pams = {}
                # per-batch intra-norm sums accumulate into one PSUM bank;
                # all sqrt/recip/broadcast/store work is deferred past the
                # last batch so the Act engine never alternates between the
                # Exp and Sqrt table sets mid-pipeline (each switch is a
                # 1.3us LoadActFuncSet)
                pnrm_all = psB.tile([1, b_loc, K], F32, name="pnrm",
                                    tag="pnrm", bufs=1)

                def s1a(b):
                    # te = assn*scale + shift on DVE; exp on Act
                    t0 = b * TPB
                    half = TPB // 2
                    te = work.tile([128, TPB, KG], F16, name="te",
                                   tag="te", bufs=2)
                    pieces = ((0, half), (half, TPB)) if b == 0 \
                        else ((0, TPB),)
                    for (ta, tb) in pieces:
                        n = tb - ta
                        nc.vector.tensor_tensor(
                            te[:, ta:tb], aqp[:, t0 + ta:t0 + tb, :KG],
                            scale_b.to_broadcast([128, n, KG]), op=OP.mult)
                        nc.vector.tensor_tensor(
                            te[:, ta:tb], te[:, ta:tb],
                            shift_b.to_broadcast([128, n, KG]), op=OP.add)
                        nc.scalar.activation(te[:, ta:tb], te[:, ta:tb],
                                             ACTF.Exp)
                    te_tiles[b] = te

                def s1b_dve(b):
                    # denominators + recip on DVE (per half, so the first
                    # renorms start earlier; GPSIMD can only reduce the
                    # partition axis, so these can't move off DVE)
                    te = te_tiles[b]
                    half = TPB // 2
                    denom = work.tile([128, TPB], F32, name="denom",
                                      tag="dn", bufs=2)
                    recip = work.tile([128, TPB], F32, name="recip",
                                      tag="rc", bufs=2)
                    for (ta, tb) in ((0, half), (half, TPB)):
                        nc.vector.tensor_reduce(
                            denom[:, ta:tb], te[:, ta:tb], axis=AX.X,
                            op=OP.add)
                        nc.vector.reciprocal(recip[:, ta:tb],
                                             denom[:, ta:tb])
                    return recip

                def s1b_renorm(b, recip):
                    # first 4 tiles on DVE (they gate the vlad start),
                    # the rest on Pool
                    t0 = b * TPB
                    te = te_tiles.pop(b)
                    half = TPB // 2
                    with nc.allow_low_precision("fp16 softmax"):
                        nc.vector.tensor_tensor(
                            sm[:, t0:t0 + 4, :], te[:, 0:4, :K],
                            recip[:, 0:4]
                            .rearrange("p (t a) -> p t a", a=1)
                            .to_broadcast([128, 4, K]), op=OP.mult)
                        for (ta, tb) in ((4, half), (half, TPB)):
                            nc.gpsimd.tensor_tensor(
                                sm[:, t0 + ta:t0 + tb, :],
                                te[:, ta:tb, :K],
                                recip[:, ta:tb]
                                .rearrange("p (t a) -> p t a", a=1)
                                .to_broadcast([128, tb - ta, K]),
                                op=OP.mult)

                def s2(b):
                    # vlad matmul with x stationary; the a_sum ones-matmuls
                    # go after the first chunk pass (all sm tiles consumed
                    # by then)
                    t0 = b * TPB
                    pas = psB.tile([1, K], F32, name="pas",
                                   tag="pas", bufs=1)
                    pv = psB.tile([128, 4, K], F32, name="pv",
                                  tag="pv", bufs=3)
                    for c in range(4):
                        for i in range(TPB):
                            src = xq[4 * b + i // QT]
                            nc.tensor.matmul(
                                pv[:, c, :],
                                src[:, i % QT, c * 128:(c + 1) * 128],
                                sm[:, t0 + i, :],
                                start=(i == 0), stop=(i == TPB - 1),
                                skip_group_check=True)
                        if c == 0:
                            for u in range(TPB):
                                nc.tensor.matmul(
                                    pas[:], ones16[:], sm[:, t0 + u, :],
                                    start=(u == 0), stop=(u == TPB - 1),
                                    skip_group_check=True)
                    return pv, pas

                def pam_bcast(b, pas):
                    nc.scalar.copy(pa_sb[:, b, :], pas[:])
                    pam = psB.tile([128, K], F32, name="pam",
                                   tag="pam", bufs=1)
                    nc.tensor.matmul(pam[:], ones_row16[:], pa_sb[:, b, :],
                                     start=True, stop=True,
                                     skip_group_check=True)
                    return pam

                def vv_tail(b, pv, pam, av_on_pool=True):
                    # vv = pv - a_sum*clusters2; squares + intra-norm sums
                    av = work.tile([128, 4, K], F32, name="av",
                                   tag="av", bufs=2)
                    if av_on_pool:
                        # GPSIMD cannot read PSUM: stage pam through SBUF
                        pam_sb = work.tile([128, K], F32, name="pam_sb",
                                           tag="pams", bufs=2)
                        nc.scalar.copy(pam_sb[:], pam[:])
                        nc.gpsimd.tensor_tensor(
                            av[:], c2n[:],
                            pam_sb[:].rearrange("p (a k) -> p a k", a=1)
                            .to_broadcast([128, 4, K]), op=OP.mult)
                    else:
                        # last batch: DVE reads pam straight from PSUM
                        # (shortest chain)
                        nc.vector.tensor_tensor(
                            av[:], c2n[:],
                            pam[:].rearrange("p (a k) -> p a k", a=1)
                            .to_broadcast([128, 4, K]), op=OP.mult)
                    nc.vector.tensor_tensor(vv[:, b], pv[:], av[:],
                                            op=OP.subtract)
                    sq = work.tile([128, 4, K], F16, name="sq",
                                   tag="sq", bufs=2)
                    with nc.allow_low_precision("fp16 vlad squares"):
                        nc.gpsimd.tensor_tensor(sq[:], vv[:, b], vv[:, b],
                                                op=OP.mult)
                    for c in range(4):
                        nc.tensor.matmul(pnrm_all[:, b, :], ones16[:],
                                         sq[:, c, :], start=(c == 0),
                                         stop=(c == 3),
                                         skip_group_check=True)

                def norm_group(b0, b1, last=False):
                    # deferred intra-norm for batches [b0, b1): sqrt of the
                    # accumulated column sums, reciprocal, one broadcast
                    # matmul, then vf + y stores. Deferring past the last
                    # Exp means a single Sqrt-table load, hidden in Act
                    # idle time. Global L2 norm of the flattened
                    # intra-normalized vlad is exactly sqrt(K)=8, folded
                    # via sqrt(64*n2).
                    nb = b1 - b0
                    snorm = work.tile([1, nb * K], F32, name="snorm",
                                      tag="n2", bufs=2)
                    nc.scalar.activation(
                        snorm[:],
                        pnrm_all[:, b0:b1].rearrange("p b k -> p (b k)"),
                        ACTF.Sqrt, bias=epsc[:], scale=64.0)
                    rn = work.tile([1, nb, K], F16, name="rn", tag="rn",
                                   bufs=2)
                    with nc.allow_low_precision("fp16 intra-norm recip"):
                        nc.vector.reciprocal(
                            rn[:].rearrange("p b k -> p (b k)"), snorm[:])
                    prnB = psB.tile([128, nb, K], F32, name="prnB",
                                    tag="misc", bufs=2)
                    nc.tensor.matmul(
                        prnB[:], ones_row16[:],
                        rn[:].rearrange("p b k -> p (b k)"),
                        start=True, stop=True, skip_group_check=True)
                    for b in range(b0, b1):
                        pieces = ((0, 2), (2, 4)) if (last and b == b1 - 1) \
                            else ((0, 4),)
                        for (ca, cb) in pieces:
                            nc.vector.tensor_tensor(
                                vf[:, b, ca:cb], vv[:, b, ca:cb],
                                prnB[:, b - b0:b - b0 + 1, :]
                                .to_broadcast([128, cb - ca, K]),
                                op=OP.mult)
                            nc.sync.dma_start(yb[:, b, ca:cb],
                                              vf[:, b, ca:cb])

                # software pipeline. Per-iteration emission order is tuned
                # per engine queue: DVE does the current batch's
                # denominators first (they gate everything downstream),
                # and the tiny pam broadcast precedes the vlad matmuls on
                # the PE. Batches 0-1 store during batch 3's vlad work;
                # batches 2-3 finish on a short final chain.
                s1a(0)
                for b in range(b_loc):
                    recip = s1b_dve(b)
                    if b >= 1:
                        pams[b - 1] = pam_bcast(b - 1, pvs[b - 1][1])
                    if b == b_loc - 1:
                        norm_group(0, 2)
                    if b + 1 < b_loc:
                        s1a(b + 1)
                    s1b_renorm(b, recip)
                    pvs[b] = s2(b)
                    if b >= 1:
                        pv_prev = pvs.pop(b - 1)[0]
                        vv_tail(b - 1, pv_prev, pams.pop(b - 1),
                                av_on_pool=True)
                bl = b_loc - 1
                pam_l = pam_bcast(bl, pvs[bl][1])
                vv_tail(bl, pvs.pop(bl)[0], pam_l, av_on_pool=False)
                norm_group(2, b_loc, last=True)
    nc.compile()
    return nc


_CACHE = {}


def _get(b_loc, n_cores, with_collective):
    key = (b_loc, n_cores, with_collective)
    if key not in _CACHE:
        _CACHE[key] = build(b_loc, n_cores, with_collective)
    return _CACHE[key]


def make_in_maps(x, clusters, clusters2, bn_gamma, bn_beta, n_cores=N_CORES):
    B = x.shape[0]
    b_loc = B // n_cores
    shared = {
        "clusters": np.ascontiguousarray(clusters, np.float32),
        "clusters2": np.ascontiguousarray(
            np.asarray(clusters2).reshape(D, K), np.float32),
        "bn_gamma": np.ascontiguousarray(
            np.asarray(bn_gamma).reshape(1, KG), np.float32),
        "bn_beta": np.ascontiguousarray(
            np.asarray(bn_beta).reshape(1, KG), np.float32),
    }
    in_maps = []
    for i in range(n_cores):
        m = dict(shared)
        m["x"] = np.ascontiguousarray(
            np.asarray(x[i * b_loc:(i + 1) * b_loc]).reshape(
                b_loc * N_SEQ, D), np.float32)
        in_maps.append(m)
    return in_maps


def kernel(x, clusters, clusters2, bn_gamma, bn_beta):
    B, N, Dd = x.shape
    assert (N, Dd) == (N_SEQ, D) and B % N_CORES == 0
    b_loc = B // N_CORES
    nc = _get(b_loc, N_CORES, True)
    in_maps = make_in_maps(x, clusters, clusters2, bn_gamma, bn_beta)
    res = run_bass_kernel_spmd(nc, in_maps, core_ids=list(range(N_CORES)))
    out = np.concatenate([res.results[i]["y"] for i in range(N_CORES)], axis=0)
    return out


# revision 48
# speedup vs baseline: 1.4605x; 1.0328x over previous
"""NetVLAD-style vq_codebook kernel for 8 Trainium2 NeuronCores.

Reference computation (per full input):
  assn = BN(x @ clusters); softmax over 80 clusters, drop 16 ghosts
  vlad[b,d,k] = sum_n assn[b,n,k] x[b,n,d] - a_sum[b,k]*clusters2[d,k]
  intra-normalize over d, flatten, global L2 normalize -> (B, D*K)

Sharding: data-parallel over batch B (B/8 batches per core). BatchNorm
statistics (sum and sum-of-squares per cluster column) are all-reduced
across the 8 cores. Everything else is local.

Schedule (per core):
  Phase A (x-load paced): 16 quarter-batch cast-DMA loads of x
    (fp32->f16, token-major). Per batch, the d-major copy of x needed by
    the assignment matmul is produced on the PE (transpose-mode matmuls
    into f16 PSUM banks, 8 chunk-tiles per bank) and copied to SBUF on
    DVE/Act -- keeping the serial DMA resource free for the loads
    (XBAR transposes would cost more DMA time than the loads). The
    assignment matmuls then run per 4-tile group; logits are copied
    PSUM->SBUF as f16 on Act into aqp (81 columns: 80 logits + a ones
    column). BN stats come from one PE matmul per tile accumulating
    G = [l|1]^T [l|1]: row 80 of G is sum(l), the diagonal is sum(l^2).
  Barrier: diagonal extraction + PE transpose to a [2,81] stats row,
    DRAM round-trip AllReduce, BN affine chain, one f16 broadcast
    matmul of scale|shift to all partitions.
  Phase B: per batch softmax (DVE scale/shift muls + one Exp on Act +
    row-sum + recip; renormalize split DVE/Pool), vlad matmul with x
    stationary, a_sum ones-matmuls, a_sum*clusters2 correction on
    GPSIMD. Tail: squares on GPSIMD, intra-norm via 4 accumulating
    ones-matmuls into one PSUM tile (no single-partition reduce),
    sqrt(64*n2) folds the exact global norm sqrt(K)=8.
"""

import sys

for _p in ("/opt/trn_rl_repo", "/root/.axon_site/_ro/trn_rl_repo"):
    if _p not in sys.path:
        sys.path.insert(0, _p)

import numpy as np

import concourse.bacc as bacc
import concourse.mybir as mybir
import concourse.tile as tile
from concourse.bass_utils import run_bass_kernel_spmd
from concourse.masks import make_identity

F32 = mybir.dt.float32
F16 = mybir.dt.float16
AX = mybir.AxisListType
OP = mybir.AluOpType
ACTF = mybir.ActivationFunctionType

N_CORES = 8
D = 512
KG = 80          # clusters + ghosts
K = 64           # real clusters
N_SEQ = 2048
TPB = N_SEQ // 128   # token tiles per batch = 16
QT = 4               # token tiles per x-load quarter
BN_EPS = 1e-5


def build(b_loc=4, n_cores=N_CORES, with_collective=True):
    """Build the per-core program. b_loc = batches per core."""
    nt = b_loc * TPB                # token tiles per core = 64
    tok = nt * 128                  # tokens per core
    total_tok = tok * n_cores       # global token count for BN stats

    nc = bacc.Bacc("TRN2", target_bir_lowering=False, debug=False,
                   dynamic_dma_scratch_size=32768)

    x = nc.declare_dram_parameter("x", [tok, D], F32, isOutput=False)
    cl = nc.declare_dram_parameter("clusters", [D, KG], F32, isOutput=False)
    c2 = nc.declare_dram_parameter("clusters2", [D, K], F32, isOutput=False)
    gam = nc.declare_dram_parameter("bn_gamma", [1, KG], F32, isOutput=False)
    bet = nc.declare_dram_parameter("bn_beta", [1, KG], F32, isOutput=False)
    y = nc.declare_dram_parameter("y", [b_loc, D * K], F32, isOutput=True)

    with tile.TileContext(nc) as tc:
        with (
            tc.tile_pool(name="persist", bufs=1) as persist,
            tc.tile_pool(name="work", bufs=4) as work,
            tc.tile_pool(name="dram", bufs=1, space="DRAM") as dram,
        ):
            # ---- persistent SBUF tensors ----
            # token-major x, one tile per load quarter (exactly one DMA
            # writer per tile keeps the dependency tracker exact)
            xq = [persist.tile([128, QT, D], F16, name=f"xq{i}")
                  for i in range(nt // QT)]
            # logits + ones column so one Gram matmul yields both BN sums
            aqp = persist.tile([128, nt, KG + 1], F16, name="aqp")
            sm = persist.tile([128, nt, K], F16, name="sm")
            clh = persist.tile([128, 4, KG], F16, name="clh")
            c2n = persist.tile([128, 4, K], F32, name="c2n")
            ones16 = persist.tile([128, 1], F16, name="ones16")
            ones_row16 = persist.tile([1, 128], F16, name="ones_row16")
            ident16 = persist.tile([128, 128], F16, name="ident16")
            ident81 = persist.tile([KG + 1, KG + 1], F32, name="ident81")
            epsc = persist.tile([1, 1], F32, name="epsc")
            gamma = persist.tile([1, KG], F32, name="gamma")
            beta = persist.tile([1, KG], F32, name="beta")
            scr2 = persist.tile([KG + 1, 2], F32, name="scr2")
            stats_g = persist.tile([1, 2 * (KG + 1)], F32, name="stats_g")
            ss = persist.tile([1, 2 * KG], F32, name="ss")
            ssh = persist.tile([1, 2 * KG], F16, name="ssh")
            bcB = persist.tile([128, 2, KG], F16, name="bcB")
            vv = persist.tile([128, b_loc, 4, K], F32, name="vv")
            vf = persist.tile([128, b_loc, 4, K], F32, name="vf")
            pa_sb = persist.tile([1, b_loc, K], F16, name="pa_sb")
            dummy = persist.tile([1, 1], F32, name="dummy")

            stats_in = dram.tile([1, 2 * (KG + 1)], F32, name="stats_in")
            stats_out = dram.tile([1, 2 * (KG + 1)], F32, name="stats_out")

            # ---- x loads: 16 quarter-batch cast DMAs (only the SWDGE /
            # gpsimd queue can cast fp32->f16). Token permutation:
            # partition p holds tokens 16p..16p+15 of each batch
            # ("(b p t) d" order, order-invariant math), so every load is
            # 128 descriptors of 8KB contiguous HBM. The identities (also
            # gpsimd) slot in after the first two load issues: ready well
            # before the first transpose, without delaying the loads.
            xr = x.ap().rearrange("(b p t) d -> p b (t d)", p=128, t=TPB)

            def emit_xload(i):
                b, q = divmod(i, TPB // QT)
                nc.gpsimd.dma_start(
                    xq[i][:].rearrange("p t d -> p (t d)"),
                    xr[:, b, q * QT * D:(q + 1) * QT * D])

            emit_xload(0)
            emit_xload(1)
            make_identity(nc, ident16[:])
            make_identity(nc, ident81[:])
            for i in range(2, nt // QT):
                emit_xload(i)
            # c2n isn't needed until phase B: issue it behind the x loads
            nc.gpsimd.dma_start(
                c2n[:], c2.ap().rearrange("(c p) k -> p c k", p=128))

            # ---- constants (small, off the gpsimd/SWDGE queue). c2n is
            # not needed until phase B: its transfer goes on the scalar
            # queue but is emitted late so it never delays the x stream.
            clf = work.tile([128, 4, KG], F32, name="clf", tag="clf", bufs=1)
            nc.scalar.dma_start(gamma[:], gam[:, :])
            nc.scalar.dma_start(beta[:], bet[:, :])
            nc.scalar.dma_start(
                clf[:], cl.ap().rearrange("(c p) k -> p c k", p=128))
            with nc.allow_low_precision("fp16 clusters"):
                nc.vector.tensor_copy(clh[:], clf[:])
            nc.vector.memset(ones16[:], 1.0)
            nc.vector.memset(ones_row16[:], 1.0)
            nc.vector.memset(epsc[:], BN_EPS)
            # ones column of aqp (stride-81 writes)
            nc.vector.memset(aqp[:, :, KG:], 1.0)
            # preload Act tables (Sqrt + Exp) while Act is idle
            nc.vector.memset(dummy[:], 1.0)
            nc.scalar.sqrt(dummy[:], dummy[:])
            nc.scalar.activation(dummy[:], dummy[:], ACTF.Exp)

            with tc.tile_pool(name="psA", bufs=1, space="PSUM") as psA:
                g81 = psA.tile([KG + 1, KG + 1], F32, name="g81",
                               tag="g81", bufs=1)

                # PE p-state warmup: data-independent matmuls fill the
                # ramp window while the first x quarter is still in
                # flight, so the real transposes start at full clock
                for _w in range(14):
                    pw = psA.tile([128, 4, KG], F32, name="pw",
                                  tag="p1", bufs=3)
                    nc.tensor.matmul(pw[:, 0, :], ident16[:],
                                     ident16[:, :KG], start=True,
                                     stop=True, skip_group_check=True)

                gram_pend = []

                def emit_gram(flush=False):
                    # lag one group behind the aqp copies so the PE never
                    # stalls on the Act copy of the current group
                    while len(gram_pend) > (0 if flush else 4):
                        t = gram_pend.pop(0)
                        nc.tensor.matmul(
                            g81[:], aqp[:, t, :], aqp[:, t, :],
                            start=(t == 0), stop=(t == nt - 1),
                            skip_group_check=True)

                def assn_group(b, g):
                    # logits for tiles 4g..4g+3 of batch b
                    p1 = psA.tile([128, 4, KG], F32, name="p1",
                                  tag="p1", bufs=3)
                    for j in range(4):
                        t = 4 * g + j
                        for c in range(4):
                            nc.tensor.matmul(
                                p1[:, j, :],
                                xhT_cur[:, 4 * t + c, :],
                                clh[:, c, :], start=(c == 0),
                                stop=(c == 3), skip_group_check=True)
                    emit_gram()
                    t0 = TPB * b + 4 * g
                    nc.scalar.copy(aqp[:, t0:t0 + 4, :KG], p1[:])
                    gram_pend.extend(range(t0, t0 + 4))

                # Per batch: 64 PE transpose-matmuls (f16 -> PSUM, 8 chunk
                # tiles per bank), bank copies to SBUF on DVE/Act, then
                # the assignment matmuls in 4-tile groups interleaved so
                # the PE keeps running while copies drain.
                for b in range(b_loc):
                    last_b = b == b_loc - 1
                    xhT_cur = work.tile([128, 4 * TPB, 128], F16,
                                        name="xhT", tag="xhT", bufs=2)
                    for qb in range(8):          # 8 PSUM banks per batch
                        tp = psA.tile([128, 8, 128], F16, name="tp",
                                      tag="tp", bufs=4)
                        for j in range(2):       # 2 token tiles per bank
                            t = 2 * qb + j
                            src = xq[4 * b + t // QT]
                            for c in range(4):
                                nc.tensor.transpose(
                                    tp[:, 4 * j + c, :],
                                    src[:, t % QT, 128 * c:128 * (c + 1)],
                                    ident16[:])
                        # DVE copies are 1.6x faster; Act takes one bank
                        # per batch to keep DVE under the load pace. For
                        # the last batch the split is even so both engines
                        # drain in parallel after the final PE transposes.
                        on_act = (qb in (1, 3, 5)) if last_b \
                            else (qb in (2, 5))
                        if on_act:
                            nc.scalar.copy(
                                xhT_cur[:, 8 * qb:8 * qb + 8, :], tp[:])
                        else:
                            nc.vector.tensor_copy(
                                xhT_cur[:, 8 * qb:8 * qb + 8, :], tp[:])
                        # interleave logits with transposes, except for
                        # the last batch: its PE stream must not stall on
                        # copies before the final transposes are emitted
                        if not last_b:
                            if qb == 3:
                                assn_group(b, 0)
                            elif qb == 5:
                                assn_group(b, 1)
                            elif qb == 7:
                                assn_group(b, 2)
                    if last_b:
                        assn_group(b, 0)
                        assn_group(b, 1)
                        assn_group(b, 2)
                    assn_group(b, 3)
                emit_gram(flush=True)

                # ---- BN stats: diag(G) = sum(l^2), row 80 = sum(l) ----
                g_scr = work.tile([KG + 1, KG + 1], F32, name="g_scr",
                                  tag="gscr", bufs=1)
                nc.vector.scalar_tensor_tensor(
                    g_scr[:], g81[:], 1.0, ident81[:],
                    op0=OP.bypass, op1=OP.mult, accum_out=scr2[:, 0:1])
                nc.vector.tensor_copy(scr2[:, 1:2], g81[:, KG:])

            with tc.tile_pool(name="psB", bufs=1, space="PSUM") as psB:
                # ---- all-reduce stats: ship the [81, 2] column-pair tile
                # straight to DRAM (k-major (sum l^2, sum l) pairs); the
                # affine chain reads the returned row through stride-2
                # views, so no transpose round-trip is needed
                nc.sync.dma_start(
                    stats_in[:].rearrange("a (p s) -> p s", p=KG + 1),
                    scr2[:])
                if with_collective:
                    nc.gpsimd.collective_compute(
                        "AllReduce", OP.add,
                        replica_groups=[list(range(n_cores))],
                        ins=[stats_in.opt()], outs=[stats_out.opt()])
                else:
                    nc.sync.dma_start(stats_out[:], stats_in[:])
                nc.sync.dma_start(stats_g[:], stats_out[:])
                esq_v = stats_g[:].rearrange("a (k s) -> a k s",
                                             s=2)[:, :KG, 0]
                suml_v = stats_g[:].rearrange("a (k s) -> a k s",
                                              s=2)[:, :KG, 1]

                # ---- BN affine: scale = gamma*rsqrt(var+eps);
                #      shift = beta - mean*scale
                mean = work.tile([1, KG], F32, name="mean", tag="sv",
                                 bufs=6)
                msq = work.tile([1, KG], F32, name="msq", tag="sv", bufs=6)
                var = work.tile([1, KG], F32, name="var", tag="sv", bufs=6)
                sd = work.tile([1, KG], F32, name="sd", tag="sv", bufs=6)
                rsd = work.tile([1, KG], F32, name="rsd", tag="sv", bufs=6)
                t1 = work.tile([1, KG], F32, name="t1", tag="sv", bufs=6)
                inv_n = 1.0 / float(total_tok)
                nc.vector.tensor_scalar_mul(mean[:], suml_v, inv_n)
                nc.vector.tensor_tensor(msq[:], mean[:], mean[:], op=OP.mult)
                # var = sum(l^2)*invN - mean^2 in one fused op
                nc.vector.scalar_tensor_tensor(
                    var[:], esq_v, inv_n, msq[:],
                    op0=OP.mult, op1=OP.subtract)
                nc.scalar.activation(sd[:], var[:], ACTF.Sqrt, bias=epsc[:])
                nc.vector.reciprocal(rsd[:], sd[:])
                nc.vector.tensor_tensor(ss[:, :KG], rsd[:], gamma[:],
                                        op=OP.mult)
                nc.vector.tensor_tensor(t1[:], mean[:], ss[:, :KG],
                                        op=OP.mult)
                nc.vector.tensor_tensor(ss[:, KG:], beta[:], t1[:],
                                        op=OP.subtract)
                with nc.allow_low_precision("fp16 bn scale/shift"):
                    nc.vector.tensor_copy(ssh[:], ss[:])

                pbc = psB.tile([128, 2 * KG], F32, name="pbc",
                               tag="misc", bufs=2)
                nc.tensor.matmul(pbc[:], ones_row16[:], ssh[:], start=True,
                                 stop=True, skip_group_check=True)
                nc.vector.tensor_copy(bcB[:].rearrange("p s k -> p (s k)"),
                                      pbc[:])
                scale_b = bcB[:, 0:1, :]
                shift_b = bcB[:, 1:2, :]

                yb = y.ap().rearrange("b (c p k) -> p b c k", p=128, k=K)
                te_tiles = {}
                pvs = {}
                pams = {}
                # per-batch intra-norm sums accumulate into one PSUM bank;
                # all sqrt/recip/broadcast/store work is deferred past the
                # last batch so the Act engine never alternates between the
                # Exp and Sqrt table sets mid-pipeline (each switch is a
                # 1.3us LoadActFuncSet)
                pnrm_all = psB.tile([1, b_loc, K], F32, name="pnrm",
                                    tag="pnrm", bufs=1)

                def s1a(b):
                    # te = assn*scale + shift on DVE; exp on Act
                    t0 = b * TPB
                    half = TPB // 2
                    te = work.tile([128, TPB, KG], F16, name="te",
                                   tag="te", bufs=2)
                    pieces = ((0, half), (half, TPB)) if b == 0 \
                        else ((0, TPB),)
                    for (ta, tb) in pieces:
                        n = tb - ta
                        nc.vector.tensor_tensor(
                            te[:, ta:tb], aqp[:, t0 + ta:t0 + tb, :KG],
                            scale_b.to_broadcast([128, n, KG]), op=OP.mult)
                        nc.vector.tensor_tensor(
                            te[:, ta:tb], te[:, ta:tb],
                            shift_b.to_broadcast([128, n, KG]), op=OP.add)
                        nc.scalar.activation(te[:, ta:tb], te[:, ta:tb],
                                             ACTF.Exp)
                    te_tiles[b] = te

                def s1b_dve(b):
                    # denominators + recip on DVE (per half, so the first
                    # renorms start earlier; GPSIMD can only reduce the
                    # partition axis, so these can't move off DVE)
                    te = te_tiles[b]
                    half = TPB // 2
                    denom = work.tile([128, TPB], F32, name="denom",
                                      tag="dn", bufs=2)
                    recip = work.tile([128, TPB], F32, name="recip",
                                      tag="rc", bufs=2)
                    for (ta, tb) in ((0, half), (half, TPB)):
                        nc.vector.tensor_reduce(
                            denom[:, ta:tb], te[:, ta:tb], axis=AX.X,
                            op=OP.add)
                        nc.vector.reciprocal(recip[:, ta:tb],
                                             denom[:, ta:tb])
                    return recip

                def s1b_renorm(b, recip):
                    # first 4 tiles on DVE (they gate the vlad start),
                    # the rest on Pool; last batch fully on DVE (idle by
                    # then, and its sm gates the end chain)
                    t0 = b * TPB
                    te = te_tiles.pop(b)
                    half = TPB // 2
                    if b == b_loc - 1:
                        pieces = ((0, half, nc.vector),
                                  (half, TPB, nc.vector))
                    else:
                        pieces = ((0, 4, nc.vector),
                                  (4, half, nc.gpsimd),
                                  (half, TPB, nc.gpsimd))
                    with nc.allow_low_precision("fp16 softmax"):
                        for (ta, tb, eng) in pieces:
                            eng.tensor_tensor(
                                sm[:, t0 + ta:t0 + tb, :],
                                te[:, ta:tb, :K],
                                recip[:, ta:tb]
                                .rearrange("p (t a) -> p t a", a=1)
                                .to_broadcast([128, tb - ta, K]),
                                op=OP.mult)

                def s2(b):
                    # vlad matmul with x stationary; the a_sum ones-matmuls
                    # go after the first chunk pass (all sm tiles consumed
                    # by then)
                    t0 = b * TPB
                    pas = psB.tile([1, K], F32, name="pas",
                                   tag="pas", bufs=1)
                    pv = psB.tile([128, 4, K], F32, name="pv",
                                  tag="pv", bufs=3)
                    for c in range(4):
                        for i in range(TPB):
                            src = xq[4 * b + i // QT]
                            nc.tensor.matmul(
                                pv[:, c, :],
                                src[:, i % QT, c * 128:(c + 1) * 128],
                                sm[:, t0 + i, :],
                                start=(i == 0), stop=(i == TPB - 1),
                                skip_group_check=True)
                        if c == 0:
                            for u in range(TPB):
                                nc.tensor.matmul(
                                    pas[:], ones16[:], sm[:, t0 + u, :],
                                    start=(u == 0), stop=(u == TPB - 1),
                                    skip_group_check=True)
                    return pv, pas

                def pam_bcast(b, pas):
                    nc.scalar.copy(pa_sb[:, b, :], pas[:])
                    pam = psB.tile([128, K], F32, name="pam",
                                   tag="pam", bufs=1)
                    nc.tensor.matmul(pam[:], ones_row16[:], pa_sb[:, b, :],
                                     start=True, stop=True,
                                     skip_group_check=True)
                    return pam

                def vv_tail(b, pv, pam, av_on_pool=True):
                    # vv = pv - a_sum*clusters2; squares + intra-norm sums
                    av = work.tile([128, 4, K], F32, name="av",
                                   tag="av", bufs=2)
                    if av_on_pool:
                        # GPSIMD cannot read PSUM: stage pam through SBUF
                        pam_sb = work.tile([128, K], F32, name="pam_sb",
                                           tag="pams", bufs=2)
                        nc.scalar.copy(pam_sb[:], pam[:])
                        nc.gpsimd.tensor_tensor(
                            av[:], c2n[:],
                            pam_sb[:].rearrange("p (a k) -> p a k", a=1)
                            .to_broadcast([128, 4, K]), op=OP.mult)
                    else:
                        # last batch: DVE reads pam straight from PSUM
                        # (shortest chain)
                        nc.vector.tensor_tensor(
                            av[:], c2n[:],
                            pam[:].rearrange("p (a k) -> p a k", a=1)
                            .to_broadcast([128, 4, K]), op=OP.mult)
                    nc.vector.tensor_tensor(vv[:, b], pv[:], av[:],
                                            op=OP.subtract)
                    sq = work.tile([128, 4, K], F16, name="sq",
                                   tag="sq", bufs=2)
                    sq_eng = nc.gpsimd if av_on_pool else nc.vector
                    with nc.allow_low_precision("fp16 vlad squares"):
                        sq_eng.tensor_tensor(sq[:], vv[:, b], vv[:, b],
                                             op=OP.mult)
                    for c in range(4):
                        nc.tensor.matmul(pnrm_all[:, b, :], ones16[:],
                                         sq[:, c, :], start=(c == 0),
                                         stop=(c == 3),
                                         skip_group_check=True)

                def norm_group(b0, b1, last=False):
                    # deferred intra-norm for batches [b0, b1): sqrt of the
                    # accumulated column sums, reciprocal, one broadcast
                    # matmul, then vf + y stores. Deferring past the last
                    # Exp means a single Sqrt-table load, hidden in Act
                    # idle time. Global L2 norm of the flattened
                    # intra-normalized vlad is exactly sqrt(K)=8, folded
                    # via sqrt(64*n2).
                    nb = b1 - b0
                    snorm = work.tile([1, nb * K], F32, name="snorm",
                                      tag="n2", bufs=2)
                    nc.scalar.activation(
                        snorm[:],
                        pnrm_all[:, b0:b1].rearrange("p b k -> p (b k)"),
                        ACTF.Sqrt, bias=epsc[:], scale=64.0)
                    rn = work.tile([1, nb, K], F16, name="rn", tag="rn",
                                   bufs=2)
                    with nc.allow_low_precision("fp16 intra-norm recip"):
                        nc.vector.reciprocal(
                            rn[:].rearrange("p b k -> p (b k)"), snorm[:])
                    prnB = psB.tile([128, nb, K], F32, name="prnB",
                                    tag="misc", bufs=2)
                    nc.tensor.matmul(
                        prnB[:], ones_row16[:],
                        rn[:].rearrange("p b k -> p (b k)"),
                        start=True, stop=True, skip_group_check=True)
                    for b in range(b0, b1):
                        pieces = ((0, 2), (2, 4)) if (last and b == b1 - 1) \
                            else ((0, 4),)
                        for (ca, cb) in pieces:
                            nc.vector.tensor_tensor(
                                vf[:, b, ca:cb], vv[:, b, ca:cb],
                                prnB[:, b - b0:b - b0 + 1, :]
                                .to_broadcast([128, cb - ca, K]),
                                op=OP.mult)
                            nc.sync.dma_start(yb[:, b, ca:cb],
                                              vf[:, b, ca:cb])

                # software pipeline. Per-iteration emission order is tuned
                # per engine queue: DVE does the current batch's
                # denominators first (they gate everything downstream),
                # and the tiny pam broadcast precedes the vlad matmuls on
                # the PE. Batches 0-1 store during batch 3's vlad work;
                # batches 2-3 finish on a short final chain.
                s1a(0)
                for b in range(b_loc):
                    recip = s1b_dve(b)
                    if b >= 1:
                        pams[b - 1] = pam_bcast(b - 1, pvs[b - 1][1])
                    if b + 1 < b_loc:
                        s1a(b + 1)
                    s1b_renorm(b, recip)
                    pvs[b] = s2(b)
                    if b >= 1:
                        pv_prev = pvs.pop(b - 1)[0]
                        vv_tail(b - 1, pv_prev, pams.pop(b - 1),
                                av_on_pool=True)
                        if b == b_loc - 1:
                            # store batches 0..2 while b3's vlad runs
                            norm_group(0, b_loc - 1)
                bl = b_loc - 1
                pam_l = pam_bcast(bl, pvs[bl][1])
                vv_tail(bl, pvs.pop(bl)[0], pam_l, av_on_pool=False)
                norm_group(bl, b_loc, last=True)
    nc.compile()
    return nc


_CACHE = {}


def _get(b_loc, n_cores, with_collective):
    key = (b_loc, n_cores, with_collective)
    if key not in _CACHE:
        _CACHE[key] = build(b_loc, n_cores, with_collective)
    return _CACHE[key]


def make_in_maps(x, clusters, clusters2, bn_gamma, bn_beta, n_cores=N_CORES):
    B = x.shape[0]
    b_loc = B // n_cores
    shared = {
        "clusters": np.ascontiguousarray(clusters, np.float32),
        "clusters2": np.ascontiguousarray(
            np.asarray(clusters2).reshape(D, K), np.float32),
        "bn_gamma": np.ascontiguousarray(
            np.asarray(bn_gamma).reshape(1, KG), np.float32),
        "bn_beta": np.ascontiguousarray(
            np.asarray(bn_beta).reshape(1, KG), np.float32),
    }
    in_maps = []
    for i in range(n_cores):
        m = dict(shared)
        m["x"] = np.ascontiguousarray(
            np.asarray(x[i * b_loc:(i + 1) * b_loc]).reshape(
                b_loc * N_SEQ, D), np.float32)
        in_maps.append(m)
    return in_maps


def kernel(x, clusters, clusters2, bn_gamma, bn_beta):
    B, N, Dd = x.shape
    assert (N, Dd) == (N_SEQ, D) and B % N_CORES == 0
    b_loc = B // N_CORES
    nc = _get(b_loc, N_CORES, True)
    in_maps = make_in_maps(x, clusters, clusters2, bn_gamma, bn_beta)
    res = run_bass_kernel_spmd(nc, in_maps, core_ids=list(range(N_CORES)))
    out = np.concatenate([res.results[i]["y"] for i in range(N_CORES)], axis=0)
    return out


# revision 64
# speedup vs baseline: 1.4827x; 1.0152x over previous
"""NetVLAD-style vq_codebook kernel for 8 Trainium2 NeuronCores.

Reference computation (per full input):
  assn = BN(x @ clusters); softmax over 80 clusters, drop 16 ghosts
  vlad[b,d,k] = sum_n assn[b,n,k] x[b,n,d] - a_sum[b,k]*clusters2[d,k]
  intra-normalize over d, flatten, global L2 normalize -> (B, D*K)

Sharding: data-parallel over batch B (B/8 batches per core). BatchNorm
statistics (sum and sum-of-squares per cluster column) are all-reduced
across the 8 cores. Everything else is local.

Schedule (per core):
  Phase A (x-load paced): 16 quarter-batch cast-DMA loads of x
    (fp32->f16, token-major). Per batch, the d-major copy of x needed by
    the assignment matmul is produced on the PE (transpose-mode matmuls
    into f16 PSUM banks, 8 chunk-tiles per bank) and copied to SBUF on
    DVE/Act -- keeping the serial DMA resource free for the loads
    (XBAR transposes would cost more DMA time than the loads). The
    assignment matmuls then run per 4-tile group; logits are copied
    PSUM->SBUF as f16 on Act into aqp (81 columns: 80 logits + a ones
    column). BN stats come from one PE matmul per tile accumulating
    G = [l|1]^T [l|1]: row 80 of G is sum(l), the diagonal is sum(l^2).
  Barrier: diagonal extraction + PE transpose to a [2,81] stats row,
    DRAM round-trip AllReduce, BN affine chain, one f16 broadcast
    matmul of scale|shift to all partitions.
  Phase B: per batch softmax (DVE scale/shift muls + one Exp on Act +
    row-sum + recip; renormalize split DVE/Pool), vlad matmul with x
    stationary, a_sum ones-matmuls, a_sum*clusters2 correction on
    GPSIMD. Tail: squares on GPSIMD, intra-norm via 4 accumulating
    ones-matmuls into one PSUM tile (no single-partition reduce),
    sqrt(64*n2) folds the exact global norm sqrt(K)=8.
"""

import sys

for _p in ("/opt/trn_rl_repo", "/root/.axon_site/_ro/trn_rl_repo"):
    if _p not in sys.path:
        sys.path.insert(0, _p)

import numpy as np

import concourse.bacc as bacc
import concourse.mybir as mybir
import concourse.tile as tile
from concourse.bass_utils import run_bass_kernel_spmd
from concourse.masks import make_identity

F32 = mybir.dt.float32
F16 = mybir.dt.float16
AX = mybir.AxisListType
OP = mybir.AluOpType
ACTF = mybir.ActivationFunctionType

N_CORES = 8
D = 512
KG = 80          # clusters + ghosts
K = 64           # real clusters
N_SEQ = 2048
TPB = N_SEQ // 128   # token tiles per batch = 16
QT = 4               # token tiles per x-load quarter
BN_EPS = 1e-5


def build(b_loc=4, n_cores=N_CORES, with_collective=True):
    """Build the per-core program. b_loc = batches per core."""
    nt = b_loc * TPB                # token tiles per core = 64
    tok = nt * 128                  # tokens per core
    total_tok = tok * n_cores       # global token count for BN stats

    nc = bacc.Bacc("TRN2", target_bir_lowering=False, debug=False,
                   dynamic_dma_scratch_size=32768)

    x = nc.declare_dram_parameter("x", [tok, D], F32, isOutput=False)
    cl = nc.declare_dram_parameter("clusters", [D, KG], F32, isOutput=False)
    c2 = nc.declare_dram_parameter("clusters2", [D, K], F32, isOutput=False)
    gam = nc.declare_dram_parameter("bn_gamma", [1, KG], F32, isOutput=False)
    bet = nc.declare_dram_parameter("bn_beta", [1, KG], F32, isOutput=False)
    y = nc.declare_dram_parameter("y", [b_loc, D * K], F32, isOutput=True)

    with tile.TileContext(nc) as tc:
        with (
            tc.tile_pool(name="persist", bufs=1) as persist,
            tc.tile_pool(name="work", bufs=4) as work,
            tc.tile_pool(name="dram", bufs=1, space="DRAM") as dram,
        ):
            # ---- persistent SBUF tensors ----
            # token-major x, one tile per load quarter (exactly one DMA
            # writer per tile keeps the dependency tracker exact)
            xq = [persist.tile([128, QT, D], F16, name=f"xq{i}")
                  for i in range(nt // QT)]
            # logits + ones column so one Gram matmul yields both BN sums
            aqp = persist.tile([128, nt, KG + 1], F16, name="aqp")
            sm = persist.tile([128, nt, K], F16, name="sm")
            clh = persist.tile([128, 4, KG], F16, name="clh")
            c2n = persist.tile([128, 4, K], F32, name="c2n")
            ones16 = persist.tile([128, 1], F16, name="ones16")
            ones_row16 = persist.tile([1, 128], F16, name="ones_row16")
            ident16 = persist.tile([128, 128], F16, name="ident16")
            ident81 = persist.tile([KG + 1, KG + 1], F32, name="ident81")
            epsc = persist.tile([1, 1], F32, name="epsc")
            gamma = persist.tile([1, KG], F32, name="gamma")
            beta = persist.tile([1, KG], F32, name="beta")
            scr2 = persist.tile([KG + 1, 2], F32, name="scr2")
            stats_g = persist.tile([1, 2 * (KG + 1)], F32, name="stats_g")
            ss = persist.tile([1, 2 * KG], F32, name="ss")
            ssh = persist.tile([1, 2 * KG], F16, name="ssh")
            bcB = persist.tile([128, 2, KG], F16, name="bcB")
            vv = persist.tile([128, b_loc, 4, K], F32, name="vv")
            vf = persist.tile([128, b_loc, 4, K], F32, name="vf")
            pa_sb = persist.tile([1, b_loc, K], F16, name="pa_sb")
            dummy = persist.tile([1, 1], F32, name="dummy")

            stats_in = dram.tile([1, 2 * (KG + 1)], F32, name="stats_in")
            stats_out = dram.tile([1, 2 * (KG + 1)], F32, name="stats_out")

            # ---- x loads: 16 quarter-batch cast DMAs (only the SWDGE /
            # gpsimd queue can cast fp32->f16). Token permutation:
            # partition p holds tokens 16p..16p+15 of each batch
            # ("(b p t) d" order, order-invariant math), so every load is
            # 128 descriptors of 8KB contiguous HBM. The identities (also
            # gpsimd) slot in after the first two load issues: ready well
            # before the first transpose, without delaying the loads.
            xr = x.ap().rearrange("(b p t) d -> p b (t d)", p=128, t=TPB)

            def emit_xload(i):
                b, q = divmod(i, TPB // QT)
                nc.gpsimd.dma_start(
                    xq[i][:].rearrange("p t d -> p (t d)"),
                    xr[:, b, q * QT * D:(q + 1) * QT * D])

            emit_xload(0)
            emit_xload(1)
            make_identity(nc, ident16[:])
            make_identity(nc, ident81[:])
            for i in range(2, nt // QT):
                emit_xload(i)
            # c2n isn't needed until phase B: issue it behind the x loads
            nc.gpsimd.dma_start(
                c2n[:], c2.ap().rearrange("(c p) k -> p c k", p=128))

            # ---- constants (small, off the gpsimd/SWDGE queue). c2n is
            # not needed until phase B: its transfer goes on the scalar
            # queue but is emitted late so it never delays the x stream.
            clf = work.tile([128, 4, KG], F32, name="clf", tag="clf", bufs=1)
            nc.scalar.dma_start(gamma[:], gam[:, :])
            nc.scalar.dma_start(beta[:], bet[:, :])
            nc.scalar.dma_start(
                clf[:], cl.ap().rearrange("(c p) k -> p c k", p=128))
            with nc.allow_low_precision("fp16 clusters"):
                nc.vector.tensor_copy(clh[:], clf[:])
            nc.vector.memset(ones16[:], 1.0)
            nc.vector.memset(ones_row16[:], 1.0)
            nc.vector.memset(epsc[:], BN_EPS)
            # ones column of aqp (stride-81 writes)
            nc.vector.memset(aqp[:, :, KG:], 1.0)
            # preload Act tables (Sqrt + Exp) while Act is idle
            nc.vector.memset(dummy[:], 1.0)
            nc.scalar.sqrt(dummy[:], dummy[:])
            nc.scalar.activation(dummy[:], dummy[:], ACTF.Exp)

            with tc.tile_pool(name="psA", bufs=1, space="PSUM") as psA:
                g81 = psA.tile([KG + 1, KG + 1], F32, name="g81",
                               tag="g81", bufs=1)

                # PE p-state warmup: data-independent matmuls fill the
                # ramp window while the first x quarter is still in
                # flight, so the real transposes start at full clock
                for _w in range(3):
                    pw = psA.tile([128, 4, KG], F32, name="pw",
                                  tag="p1", bufs=3)
                    nc.tensor.matmul(pw[:, 0, :], ident16[:],
                                     ident16[:, :KG], start=True,
                                     stop=True, skip_group_check=True)

                gram_pend = []

                def emit_gram(flush=False):
                    # lag one group behind the aqp copies so the PE never
                    # stalls on the Act copy of the current group
                    while len(gram_pend) > (0 if flush else 4):
                        t = gram_pend.pop(0)
                        nc.tensor.matmul(
                            g81[:], aqp[:, t, :], aqp[:, t, :],
                            start=(t == 0), stop=(t == nt - 1),
                            skip_group_check=True)

                def assn_group(b, g, j0=0, nj=4):
                    # logits for tiles 4g+j0 .. 4g+j0+nj-1 of batch b
                    p1 = psA.tile([128, 4, KG], F32, name="p1",
                                  tag="p1", bufs=3)
                    for j in range(j0, j0 + nj):
                        t = 4 * g + j
                        for c in range(4):
                            nc.tensor.matmul(
                                p1[:, j, :],
                                xhT_cur[:, 4 * t + c, :],
                                clh[:, c, :], start=(c == 0),
                                stop=(c == 3), skip_group_check=True)
                    emit_gram()
                    t0 = TPB * b + 4 * g + j0
                    if b == b_loc - 1 and g == 3:
                        nc.vector.tensor_copy(
                            aqp[:, t0:t0 + nj, :KG], p1[:, j0:j0 + nj])
                    else:
                        nc.scalar.copy(aqp[:, t0:t0 + nj, :KG],
                                       p1[:, j0:j0 + nj])
                    gram_pend.extend(range(t0, t0 + nj))

                # Per batch: 64 PE transpose-matmuls (f16 -> PSUM, 8 chunk
                # tiles per bank), bank copies to SBUF on DVE/Act, then
                # the assignment matmuls in 4-tile groups interleaved so
                # the PE keeps running while copies drain.
                for b in range(b_loc):
                    last_b = b == b_loc - 1
                    xhT_cur = work.tile([128, 4 * TPB, 128], F16,
                                        name="xhT", tag="xhT", bufs=2)
                    for qb in range(8):          # 8 PSUM banks per batch
                        tp = psA.tile([128, 8, 128], F16, name="tp",
                                      tag="tp", bufs=4)
                        for j in range(2):       # 2 token tiles per bank
                            t = 2 * qb + j
                            src = xq[4 * b + t // QT]
                            for c in range(4):
                                nc.tensor.transpose(
                                    tp[:, 4 * j + c, :],
                                    src[:, t % QT, 128 * c:128 * (c + 1)],
                                    ident16[:])
                        # DVE copies are 1.6x faster; Act takes one bank
                        # per batch to keep DVE under the load pace. For
                        # the last batch the split is even so both engines
                        # drain in parallel after the final PE transposes.
                        on_act = (qb in (1, 3)) if last_b \
                            else (qb in (2, 5))
                        if on_act:
                            nc.scalar.copy(
                                xhT_cur[:, 8 * qb:8 * qb + 8, :], tp[:])
                        else:
                            nc.vector.tensor_copy(
                                xhT_cur[:, 8 * qb:8 * qb + 8, :], tp[:])
                        # interleave logits with transposes, except for
                        # the last batch: its PE stream must not stall on
                        # copies before the final transposes are emitted
                        if not last_b:
                            if qb == 3:
                                assn_group(b, 0)
                            elif qb == 5:
                                assn_group(b, 1)
                            elif qb == 7:
                                assn_group(b, 2)
                    if last_b:
                        assn_group(b, 0)
                        assn_group(b, 1)
                        assn_group(b, 2)
                        assn_group(b, 3, j0=0, nj=2)
                        assn_group(b, 3, j0=2, nj=2)
                    else:
                        assn_group(b, 3)
                    if b == b_loc - 2:
                        # pull the barrier Sqrt's table load into Act
                        # slack here rather than the post-load drain
                        nc.scalar.sqrt(dummy[:], dummy[:])
                emit_gram(flush=True)

                # ---- BN stats: diag(G) = sum(l^2), row 80 = sum(l) ----
                g_scr = work.tile([KG + 1, KG + 1], F32, name="g_scr",
                                  tag="gscr", bufs=1)
                nc.vector.scalar_tensor_tensor(
                    g_scr[:], g81[:], 1.0, ident81[:],
                    op0=OP.bypass, op1=OP.mult, accum_out=scr2[:, 0:1])
                nc.vector.tensor_copy(scr2[:, 1:2], g81[:, KG:])

            with tc.tile_pool(name="psB", bufs=1, space="PSUM") as psB:
                # ---- all-reduce stats: ship the [81, 2] column-pair tile
                # straight to DRAM (k-major (sum l^2, sum l) pairs); the
                # affine chain reads the returned row through stride-2
                # views, so no transpose round-trip is needed
                nc.sync.dma_start(
                    stats_in[:].rearrange("a (p s) -> p s", p=KG + 1),
                    scr2[:])
                if with_collective:
                    nc.gpsimd.collective_compute(
                        "AllReduce", OP.add,
                        replica_groups=[list(range(n_cores))],
                        ins=[stats_in.opt()], outs=[stats_out.opt()])
                else:
                    nc.sync.dma_start(stats_out[:], stats_in[:])
                nc.sync.dma_start(stats_g[:], stats_out[:])
                esq_v = stats_g[:].rearrange("a (k s) -> a k s",
                                             s=2)[:, :KG, 0]
                suml_v = stats_g[:].rearrange("a (k s) -> a k s",
                                              s=2)[:, :KG, 1]

                # ---- BN affine: scale = gamma*rsqrt(var+eps);
                #      shift = beta - (mean*gamma)*rsqrt(var+eps).
                # mg = mean*gamma branches off early so the post-rsd chain
                # is two ops; scale/shift write the f16 row directly.
                msq = work.tile([1, KG], F32, name="msq", tag="sv", bufs=6)
                mg = work.tile([1, KG], F32, name="mg", tag="sv", bufs=6)
                var = work.tile([1, KG], F32, name="var", tag="sv", bufs=6)
                sd = work.tile([1, KG], F32, name="sd", tag="sv", bufs=6)
                rsd = work.tile([1, KG], F32, name="rsd", tag="sv", bufs=6)
                t1 = work.tile([1, KG], F32, name="t1", tag="sv", bufs=6)
                inv_n = 1.0 / float(total_tok)
                # msq = (sum(l)*invN)^2 and mg = mean*gamma, each in one
                # fused op (no separate mean)
                nc.vector.scalar_tensor_tensor(
                    msq[:], suml_v, inv_n * inv_n, suml_v,
                    op0=OP.mult, op1=OP.mult)
                nc.vector.scalar_tensor_tensor(
                    mg[:], suml_v, inv_n, gamma[:], op0=OP.mult,
                    op1=OP.mult)
                # var = sum(l^2)*invN - mean^2 in one fused op
                nc.vector.scalar_tensor_tensor(
                    var[:], esq_v, inv_n, msq[:],
                    op0=OP.mult, op1=OP.subtract)
                nc.scalar.activation(sd[:], var[:], ACTF.Sqrt, bias=epsc[:])
                nc.vector.reciprocal(rsd[:], sd[:])
                with nc.allow_low_precision("fp16 bn scale/shift"):
                    nc.vector.tensor_tensor(ssh[:, :KG], rsd[:], gamma[:],
                                            op=OP.mult)
                nc.vector.tensor_tensor(t1[:], mg[:], rsd[:], op=OP.mult)
                with nc.allow_low_precision("fp16 bn scale/shift"):
                    nc.vector.tensor_tensor(ssh[:, KG:], beta[:], t1[:],
                                            op=OP.subtract)

                # broadcast scale first: the first softmax multiply only
                # needs the scale half
                pbc1 = psB.tile([128, KG], F32, name="pbc1",
                                tag="misc", bufs=2)
                nc.tensor.matmul(pbc1[:], ones_row16[:], ssh[:, :KG],
                                 start=True, stop=True,
                                 skip_group_check=True)
                nc.vector.tensor_copy(bcB[:, 0, :], pbc1[:])
                pbc2 = psB.tile([128, KG], F32, name="pbc2",
                                tag="misc", bufs=2)
                nc.tensor.matmul(pbc2[:], ones_row16[:], ssh[:, KG:],
                                 start=True, stop=True,
                                 skip_group_check=True)
                nc.vector.tensor_copy(bcB[:, 1, :], pbc2[:])
                scale_b = bcB[:, 0:1, :]
                shift_b = bcB[:, 1:2, :]

                yb = y.ap().rearrange("b (c p k) -> p b c k", p=128, k=K)
                te_tiles = {}
                pvs = {}
                pams = {}
                # per-batch intra-norm sums accumulate into one PSUM bank;
                # all sqrt/recip/broadcast/store work is deferred past the
                # last batch so the Act engine never alternates between the
                # Exp and Sqrt table sets mid-pipeline (each switch is a
                # 1.3us LoadActFuncSet)
                pnrm_all = psB.tile([1, b_loc, K], F32, name="pnrm",
                                    tag="pnrm", bufs=1)

                def s1a(b):
                    # te = assn*scale + shift on DVE; exp on Act
                    t0 = b * TPB
                    half = TPB // 2
                    te = work.tile([128, TPB, KG], F16, name="te",
                                   tag="te", bufs=2)
                    pieces = ((0, half), (half, TPB)) if b == 0 \
                        else ((0, TPB),)
                    for (ta, tb) in pieces:
                        n = tb - ta
                        nc.vector.tensor_tensor(
                            te[:, ta:tb], aqp[:, t0 + ta:t0 + tb, :KG],
                            scale_b.to_broadcast([128, n, KG]), op=OP.mult)
                        nc.vector.tensor_tensor(
                            te[:, ta:tb], te[:, ta:tb],
                            shift_b.to_broadcast([128, n, KG]), op=OP.add)
                        nc.scalar.activation(te[:, ta:tb], te[:, ta:tb],
                                             ACTF.Exp)
                    te_tiles[b] = te

                def s1b_dve(b):
                    # denominators + recip on DVE (per half, so the first
                    # renorms start earlier; GPSIMD can only reduce the
                    # partition axis, so these can't move off DVE)
                    te = te_tiles[b]
                    half = TPB // 2
                    denom = work.tile([128, TPB], F32, name="denom",
                                      tag="dn", bufs=2)
                    recip = work.tile([128, TPB], F32, name="recip",
                                      tag="rc", bufs=2)
                    for (ta, tb) in ((0, half), (half, TPB)):
                        nc.vector.tensor_reduce(
                            denom[:, ta:tb], te[:, ta:tb], axis=AX.X,
                            op=OP.add)
                        nc.vector.reciprocal(recip[:, ta:tb],
                                             denom[:, ta:tb])
                    return recip

                def s1b_renorm(b, recip):
                    # first 4 tiles on DVE (they gate the vlad start),
                    # the rest on Pool; last batch fully on DVE (idle by
                    # then, and its sm gates the end chain)
                    t0 = b * TPB
                    te = te_tiles.pop(b)
                    half = TPB // 2
                    if b == b_loc - 1:
                        pieces = ((0, half, nc.vector),
                                  (half, TPB, nc.vector))
                    else:
                        pieces = ((0, 4, nc.vector),
                                  (4, half, nc.gpsimd),
                                  (half, TPB, nc.gpsimd))
                    with nc.allow_low_precision("fp16 softmax"):
                        for (ta, tb, eng) in pieces:
                            eng.tensor_tensor(
                                sm[:, t0 + ta:t0 + tb, :],
                                te[:, ta:tb, :K],
                                recip[:, ta:tb]
                                .rearrange("p (t a) -> p t a", a=1)
                                .to_broadcast([128, tb - ta, K]),
                                op=OP.mult)

                def s2(b):
                    # vlad matmul with x stationary; the a_sum ones-matmuls
                    # go after the first chunk pass (all sm tiles consumed
                    # by then)
                    t0 = b * TPB
                    pas = psB.tile([1, K], F32, name="pas",
                                   tag="pas", bufs=1)
                    pv = psB.tile([128, 4, K], F32, name="pv",
                                  tag="pv", bufs=3)
                    for c in range(4):
                        for i in range(TPB):
                            src = xq[4 * b + i // QT]
                            nc.tensor.matmul(
                                pv[:, c, :],
                                src[:, i % QT, c * 128:(c + 1) * 128],
                                sm[:, t0 + i, :],
                                start=(i == 0), stop=(i == TPB - 1),
                                skip_group_check=True)
                        if c == 0:
                            for u in range(TPB):
                                nc.tensor.matmul(
                                    pas[:], ones16[:], sm[:, t0 + u, :],
                                    start=(u == 0), stop=(u == TPB - 1),
                                    skip_group_check=True)
                    return pv, pas

                def pam_bcast(b, pas):
                    nc.scalar.copy(pa_sb[:, b, :], pas[:])
                    pam = psB.tile([128, K], F32, name="pam",
                                   tag="pam", bufs=1)
                    nc.tensor.matmul(pam[:], ones_row16[:], pa_sb[:, b, :],
                                     start=True, stop=True,
                                     skip_group_check=True)
                    return pam

                def vv_tail(b, pv, pam, av_on_pool=True):
                    # vv = pv - a_sum*clusters2; squares + intra-norm sums
                    av = work.tile([128, 4, K], F32, name="av",
                                   tag="av", bufs=2)
                    if av_on_pool:
                        # GPSIMD cannot read PSUM: stage pam through SBUF
                        pam_sb = work.tile([128, K], F32, name="pam_sb",
                                           tag="pams", bufs=2)
                        nc.scalar.copy(pam_sb[:], pam[:])
                        nc.gpsimd.tensor_tensor(
                            av[:], c2n[:],
                            pam_sb[:].rearrange("p (a k) -> p a k", a=1)
                            .to_broadcast([128, 4, K]), op=OP.mult)
                    else:
                        # last batch: DVE reads pam straight from PSUM
                        # (shortest chain)
                        nc.vector.tensor_tensor(
                            av[:], c2n[:],
                            pam[:].rearrange("p (a k) -> p a k", a=1)
                            .to_broadcast([128, 4, K]), op=OP.mult)
                    nc.vector.tensor_tensor(vv[:, b], pv[:], av[:],
                                            op=OP.subtract)
                    sq = work.tile([128, 4, K], F16, name="sq",
                                   tag="sq", bufs=2)
                    sq_eng = nc.gpsimd if av_on_pool else nc.vector
                    with nc.allow_low_precision("fp16 vlad squares"):
                        sq_eng.tensor_tensor(sq[:], vv[:, b], vv[:, b],
                                             op=OP.mult)
                    for c in range(4):
                        nc.tensor.matmul(pnrm_all[:, b, :], ones16[:],
                                         sq[:, c, :], start=(c == 0),
                                         stop=(c == 3),
                                         skip_group_check=True)

                def norm_group(b0, b1, last=False):
                    # deferred intra-norm for batches [b0, b1): sqrt of the
                    # accumulated column sums, reciprocal, one broadcast
                    # matmul, then vf + y stores. Deferring past the last
                    # Exp means a single Sqrt-table load, hidden in Act
                    # idle time. Global L2 norm of the flattened
                    # intra-normalized vlad is exactly sqrt(K)=8, folded
                    # via sqrt(64*n2).
                    nb = b1 - b0
                    snorm = work.tile([1, nb * K], F32, name="snorm",
                                      tag="n2", bufs=2)
                    nc.scalar.activation(
                        snorm[:],
                        pnrm_all[:, b0:b1].rearrange("p b k -> p (b k)"),
                        ACTF.Sqrt, bias=epsc[:], scale=64.0)
                    rn = work.tile([1, nb, K], F16, name="rn", tag="rn",
                                   bufs=2)
                    with nc.allow_low_precision("fp16 intra-norm recip"):
                        nc.vector.reciprocal(
                            rn[:].rearrange("p b k -> p (b k)"), snorm[:])
                    prnB = psB.tile([128, nb, K], F32, name="prnB",
                                    tag="misc", bufs=2)
                    nc.tensor.matmul(
                        prnB[:], ones_row16[:],
                        rn[:].rearrange("p b k -> p (b k)"),
                        start=True, stop=True, skip_group_check=True)
                    for b in range(b0, b1):
                        pieces = ((0, 2), (2, 4)) if (last and b == b1 - 1) \
                            else ((0, 4),)
                        for (ca, cb) in pieces:
                            nc.vector.tensor_tensor(
                                vf[:, b, ca:cb], vv[:, b, ca:cb],
                                prnB[:, b - b0:b - b0 + 1, :]
                                .to_broadcast([128, cb - ca, K]),
                                op=OP.mult)
                            nc.sync.dma_start(yb[:, b, ca:cb],
                                              vf[:, b, ca:cb])

                # software pipeline. Per-iteration emission order is tuned
                # per engine queue: DVE does the current batch's
                # denominators first (they gate everything downstream),
                # and the tiny pam broadcast precedes the vlad matmuls on
                # the PE. Batches 0-1 store during batch 3's vlad work;
                # batches 2-3 finish on a short final chain.
                s1a(0)
                for b in range(b_loc):
                    recip = s1b_dve(b)
                    if b >= 1:
                        pams[b - 1] = pam_bcast(b - 1, pvs[b - 1][1])
                    if b + 1 < b_loc:
                        s1a(b + 1)
                    s1b_renorm(b, recip)
                    pvs[b] = s2(b)
                    if b >= 1:
                        pv_prev = pvs.pop(b - 1)[0]
                        vv_tail(b - 1, pv_prev, pams.pop(b - 1),
                                av_on_pool=True)
                        if b == b_loc - 1:
                            # store batches 0..2 while b3's vlad runs
                            norm_group(0, b_loc - 1)
                bl = b_loc - 1
                pam_l = pam_bcast(bl, pvs[bl][1])
                vv_tail(bl, pvs.pop(bl)[0], pam_l, av_on_pool=False)
                norm_group(bl, b_loc, last=True)
    nc.compile()
    return nc


_CACHE = {}


def _get(b_loc, n_cores, with_collective):
    key = (b_loc, n_cores, with_collective)
    if key not in _CACHE:
        _CACHE[key] = build(b_loc, n_cores, with_collective)
    return _CACHE[key]


def make_in_maps(x, clusters, clusters2, bn_gamma, bn_beta, n_cores=N_CORES):
    B = x.shape[0]
    b_loc = B // n_cores
    shared = {
        "clusters": np.ascontiguousarray(clusters, np.float32),
        "clusters2": np.ascontiguousarray(
            np.asarray(clusters2).reshape(D, K), np.float32),
        "bn_gamma": np.ascontiguousarray(
            np.asarray(bn_gamma).reshape(1, KG), np.float32),
        "bn_beta": np.ascontiguousarray(
            np.asarray(bn_beta).reshape(1, KG), np.float32),
    }
    in_maps = []
    for i in range(n_cores):
        m = dict(shared)
        m["x"] = np.ascontiguousarray(
            np.asarray(x[i * b_loc:(i + 1) * b_loc]).reshape(
                b_loc * N_SEQ, D), np.float32)
        in_maps.append(m)
    return in_maps


def kernel(x, clusters, clusters2, bn_gamma, bn_beta):
    B, N, Dd = x.shape
    assert (N, Dd) == (N_SEQ, D) and B % N_CORES == 0
    b_loc = B // N_CORES
    nc = _get(b_loc, N_CORES, True)
    in_maps = make_in_maps(x, clusters, clusters2, bn_gamma, bn_beta)
    res = run_bass_kernel_spmd(nc, in_maps, core_ids=list(range(N_CORES)))
    out = np.concatenate([res.results[i]["y"] for i in range(N_CORES)], axis=0)
    return out
